# revision 19
# baseline (speedup 1.0000x reference)
"""Trainium2 Bass kernel for nn_BasicBlock (distance-transform conv BasicBlock).

Computes: relu(bn2(dt_conv2(relu(bn1(dt_conv1(x))))) + x)
where dt_conv is a 3x3 "distance transform conv":
    d[b,o,h,w] = sqrt(||p - c_o||^2),  p = 3x3 zero-padded patch (dim 576)

Strategy (8 NeuronCores, data-parallel over batch 32 -> 4 images/core):
- ||p||^2 - 2 p.c in ONE matmul accumulation group per pixel tile: SBUF
  partitions 0:64 hold x (weights = -2*centers), partitions 64:128 hold x^2
  (weights = 1.0). 9 shifted matmuls (3x3 offsets) accumulate in PSUM,
  K=128, M=64 out channels, N=448 (8 rows x 56).
- PE column-pairing: images (0,2) and (1,3) share one PSUM bank — image A
  accumulates into psum[0:64] (tile_position (0,0)), image B into
  psum[64:128] ((0,64)). The two 64-col PE groups run concurrently,
  ~doubling effective matmul throughput vs M=64 alone.
- All matmul operands bf16 (tolerance 2e-2; d and BN stats stay f32 where
  conditioning demands). Halves SBUF streams + input DMA.
- Evictions 128-wide: d = sqrt(psum + ||c||^2) on ScalarE per image-PAIR,
  accum_out gives per-channel sum(d). sum(d^2) = sum(psum) + n*c2 via one
  128-wide DVE reduce per pair.
- Sync-BN: fold upper/lower halves, [64,2] AllGather across 8 cores +
  local rank-sum, x2 layers.
- Output written bf16 and converted to f32 on host; out-DMA split across
  rings to shrink the post-barrier tail.

kernel(**inputs) takes FULL unsharded inputs, returns FULL output.
Self-contained: shapes/sharding hardcoded; no file reads.
"""
import numpy as np

from concourse import bacc, mybir, tile
from concourse.bass_utils import run_bass_kernel_spmd

f32 = mybir.dt.float32
bf16 = mybir.dt.bfloat16
ADD = mybir.AluOpType.add
MULT = mybir.AluOpType.mult
SUB = mybir.AluOpType.subtract
AF = mybir.ActivationFunctionType

N_CORES = 8
B_LOCAL = 4            # images per core (32 / 8)
C = 64                 # channels (in == out)
HW = 56                # spatial
HP = HW + 2            # padded
RPG = 8                # rows per matmul group (N = 8*56 = 448)
NGRP = 7               # row-groups per image-pair slot (56 / 8)
N_GLOBAL = 32 * HW * HW
BN_EPS = 1e-5


def _pb(b):
    """Partition base and pair-slot index for the 128-wide d/xres layout."""
    return 64 * (b // 2), b % 2


def _build_layer(nc, psum, src, w, cst, ci, d, sumd, sumps, slots=(0, 1)):
    """One dt_conv layer (or one pair-slot of it). src[b] is a [128, HP, HP]
    bf16 tile (x | x^2). Image pair (i, i+2) shares a PSUM bank: i ->
    psum[0:64] (PE cols 0:64), i+2 -> psum[64:128] (PE cols 64:128), running
    concurrently. d is [128, 2, HW, HW] f32; sumd/sumps [128, 2*NGRP].
    slots selects which image pairs to emit — callers phase slot 1 after
    its glue so ACT evictions interleave with glue instead of queueing
    behind it (ACT is strict FIFO; stuck evictions stall PSUM reuse)."""
    evicts = []
    order = [(g, i) for g in range(3) for i in slots] + \
            [(g, i) for g in range(3, NGRP) for i in slots]
    for g, i in order:
        ps = psum.tile([2 * C, RPG, 64], f32, tag="ps")
        r0 = g * RPG
        for k in range(9):
            kh, kw = k // 3, k % 3
            nc.tensor.matmul(
                ps[0:C, 0:RPG, 0:HW],
                w[:, k, :],
                src[i][:, r0 + kh:r0 + kh + RPG, kw:kw + HW],
                start=(k == 0), stop=(k == 8),
            )
            nc.tensor.matmul(
                ps[C:2 * C, 0:RPG, 0:HW],
                w[:, k, :],
                src[i + 2][:, r0 + kh:r0 + kh + RPG, kw:kw + HW],
                start=(k == 0), stop=(k == 8),
            )
        col = i * NGRP + g
        # per-channel sum(psum) (-> sum(d^2) after +n*c2); emitted first so
        # it runs concurrently with the ACT eviction
        nc.vector.tensor_reduce(
            out=sumps[:, col:col + 1],
            in_=ps[:, 0:RPG, 0:HW],
            axis=mybir.AxisListType.XY, op=ADD)
        # d = sqrt(psum + ||c||^2); accum_out gives per-channel sum(d)
        ev = nc.scalar.activation(
            out=d[:, i, r0:r0 + RPG, :],
            in_=ps[:, 0:RPG, 0:HW],
            func=AF.Sqrt, bias=cst[:, ci:ci + 1], scale=1.0,
            accum_out=sumd[:, col:col + 1])
        evicts.append(ev)
    return evicts


def _build_layer_nopair(nc, psum, src, w, cst, ci, d, sumd, sumps):
    """A/B variant: no PSUM column pairing — per-image groups, M=64 at
    tile_position (0,0), 64-wide evictions (baseline-style)."""
    evicts = []
    order = [(g, b) for g in range(3) for b in range(B_LOCAL)] + \
            [(g, b) for g in range(3, NGRP) for b in range(B_LOCAL)]
    for g, b in order:
        pb, i = _pb(b)
        ps = psum.tile([C, RPG, 64], f32, tag="psn")
        r0 = g * RPG
        for k in range(9):
            kh, kw = k // 3, k % 3
            nc.tensor.matmul(
                ps[:, 0:RPG, 0:HW],
                w[:, k, :],
                src[b][:, r0 + kh:r0 + kh + RPG, kw:kw + HW],
                start=(k == 0), stop=(k == 8),
            )
        col = b * NGRP + g
        nc.vector.tensor_reduce(
            out=sumps[:, col:col + 1],
            in_=ps[:, 0:RPG, 0:HW],
            axis=mybir.AxisListType.XY, op=ADD)
        ev = nc.scalar.activation(
            out=d[pb:pb + C, i, r0:r0 + RPG, :],
            in_=ps[:, 0:RPG, 0:HW],
            func=AF.Sqrt, bias=cst[pb:pb + C, ci:ci + 1], scale=1.0,
            accum_out=sumd[:, col:col + 1])
        evicts.append(ev)
    return evicts


def _stats_allreduce_nopair(nc, pool, dram, sumd, sumps, name,
                            no_collective=False):
    red = pool.tile([C, 2], f32, tag=f"redn_{name}")
    gstats = pool.tile([2 * C, 2], f32, tag=f"gstatsn_{name}")
    nc.vector.tensor_reduce(out=red[:, 0:1], in_=sumd[:, :],
                            axis=mybir.AxisListType.X, op=ADD)
    nc.vector.tensor_reduce(out=red[:, 1:2], in_=sumps[:, :],
                            axis=mybir.AxisListType.X, op=ADD)
    if no_collective:
        nc.vector.tensor_copy(out=gstats[0:C, :], in_=red[:, :])
        nc.vector.tensor_copy(out=gstats[C:2 * C, :], in_=gstats[0:C, :])
        return gstats
    cc_in = dram.tile([C, 2], f32, tag=f"ccinn_{name}")
    cc_out = dram.tile([N_CORES * C, 2], f32, tag=f"ccoutn_{name}")
    gag = pool.tile([C, N_CORES, 2], f32, tag=f"gagn_{name}")
    nc.sync.dma_start(out=cc_in[:, :], in_=red[:, :])
    nc.gpsimd.collective_compute(
        "AllGather", mybir.AluOpType.bypass,
        replica_groups=[list(range(N_CORES))],
        ins=[cc_in.opt()],
        outs=[cc_out.opt()],
    )
    nc.sync.dma_start(
        out=gag[:, :, :],
        in_=cc_out[:, :].rearrange("(r c) s -> c r s", r=N_CORES))
    nc.vector.tensor_reduce(out=gstats[0:C, 0:1], in_=gag[:, :, 0],
                            axis=mybir.AxisListType.X, op=ADD)
    nc.vector.tensor_reduce(out=gstats[0:C, 1:2], in_=gag[:, :, 1],
                            axis=mybir.AxisListType.X, op=ADD)
    nc.vector.tensor_copy(out=gstats[C:2 * C, :], in_=gstats[0:C, :])
    return gstats


def _bn_affine(nc, pool, gstats, zc2, gamma, beta, eps, name):
    """From [sum(d), sum(psum)] (dup both halves) -> scale s, shift t [128,1].
    zc2 is a [128, 2] cst slice [zero, c2]."""
    P = 2 * C
    mued = pool.tile([P, 2], f32, tag=f"mued_{name}")
    mu2 = pool.tile([P, 1], f32, tag=f"mu2_{name}")
    var = pool.tile([P, 1], f32, tag=f"var_{name}")
    sd = pool.tile([P, 1], f32, tag=f"sd_{name}")
    inv = pool.tile([P, 1], f32, tag=f"inv_{name}")
    s = pool.tile([P, 1], f32, tag=f"s_{name}")
    st = pool.tile([P, 1], f32, tag=f"st_{name}")
    tt = pool.tile([P, 1], f32, tag=f"t_{name}")
    inv_n = 1.0 / float(N_GLOBAL)
    # [mu, E[d^2]] = gstats * 1/N + [0, c2] in one DVE op
    nc.vector.scalar_tensor_tensor(
        out=mued[:, :], in0=gstats[:, 0:2], scalar=inv_n, in1=zc2,
        op0=MULT, op1=ADD)
    mu, ed2 = mued[:, 0:1], mued[:, 1:2]
    nc.vector.tensor_tensor(out=mu2[:, :], in0=mu, in1=mu, op=MULT)
    nc.vector.tensor_tensor(out=var[:, :], in0=ed2, in1=mu2[:, :], op=SUB)
    nc.scalar.activation(out=sd[:, :], in_=var[:, :], func=AF.Sqrt,
                         bias=eps[:, 0:1], scale=1.0)
    nc.vector.reciprocal(out=inv[:, :], in_=sd[:, :])
    nc.vector.tensor_tensor(out=s[:, :], in0=gamma, in1=inv[:, :], op=MULT)
    nc.vector.tensor_tensor(out=st[:, :], in0=mu, in1=s[:, :], op=MULT)
    nc.vector.tensor_tensor(out=tt[:, :], in0=beta, in1=st[:, :], op=SUB)
    return s, tt


def _stats_allreduce(nc, pool, dram, sumd, sumps, name, no_collective=False):
    """Reduce [128, 2*NGRP] stat columns, fold upper half into lower,
    AllGather [64,2] across 8 cores + local rank-sum, return [128,2]
    duplicated global sums."""
    red = pool.tile([2 * C, 2], f32, tag=f"red_{name}")
    gstats = pool.tile([2 * C, 2], f32, tag=f"gstats_{name}")
    nc.vector.tensor_reduce(out=red[:, 0:1], in_=sumd[:, :],
                            axis=mybir.AxisListType.X, op=ADD)
    nc.vector.tensor_reduce(out=red[:, 1:2], in_=sumps[:, :],
                            axis=mybir.AxisListType.X, op=ADD)
    # images (2,3) stats live on the upper partition half; DMA both halves
    # side by side into the collective input (DVE tensor ops can't mix
    # partition bases, DMA can) and fold during the rank-sum reduce.
    cc_in = dram.tile([C, 2, 2], f32, tag=f"ccin_{name}")
    # one DMA: partition-major red [128,2] -> (h, c, s) walk of cc_in
    nc.sync.dma_start(out=cc_in[:, :, :].rearrange("c s h -> h c s"),
                      in_=red[:, :])
    if no_collective:
        gag = pool.tile([C, 2, 2], f32, tag=f"gag_{name}")
        nc.sync.dma_start(out=gag[:, :, :], in_=cc_in[:, :, :])
        nc.vector.tensor_reduce(out=gstats[0:C, 0:1], in_=gag[:, 0, :],
                                axis=mybir.AxisListType.X, op=ADD)
        nc.vector.tensor_reduce(out=gstats[0:C, 1:2], in_=gag[:, 1, :],
                                axis=mybir.AxisListType.X, op=ADD)
        nc.vector.tensor_copy(out=gstats[C:2 * C, :], in_=gstats[0:C, :])
        return gstats
    # AllGather (floor ~4.6us vs AllReduce ~9.7us) + local rank-sum.
    cc_out = dram.tile([N_CORES * C, 2, 2], f32, tag=f"ccout_{name}")
    gag = pool.tile([C, N_CORES, 2, 2], f32, tag=f"gag_{name}")
    nc.gpsimd.collective_compute(
        "AllGather", mybir.AluOpType.bypass,
        replica_groups=[list(range(N_CORES))],
        ins=[cc_in.opt()],
        outs=[cc_out.opt()],
    )
    nc.sync.dma_start(
        out=gag[:, :, :, :],
        in_=cc_out[:, :, :].rearrange("(r c) s h -> c r s h", r=N_CORES))
    nc.vector.tensor_reduce(out=gstats[0:C, 0:1], in_=gag[:, :, 0, :],
                            axis=mybir.AxisListType.XY, op=ADD)
    nc.vector.tensor_reduce(out=gstats[0:C, 1:2], in_=gag[:, :, 1, :],
                            axis=mybir.AxisListType.XY, op=ADD)
    nc.vector.tensor_copy(out=gstats[C:2 * C, :], in_=gstats[0:C, :])
    return gstats


def build(no_collective=False, reps=1, col_pair=True):
    nc = bacc.Bacc("TRN2", target_bir_lowering=False, debug=False,
                   num_devices=1 if no_collective else N_CORES)
    x_ext = nc.declare_dram_parameter("x", [B_LOCAL, C, HW, HW], bf16,
                                      isOutput=False)
    xf_ext = nc.declare_dram_parameter("xf", [B_LOCAL, C, HW, HW], f32,
                                       isOutput=False)
    xsq_ext = nc.declare_dram_parameter("xsq", [B_LOCAL, C, HW, HW], bf16,
                                        isOutput=False)
    w1_ext = nc.declare_dram_parameter("w1", [2 * C, 9, C], bf16, isOutput=False)
    w2_ext = nc.declare_dram_parameter("w2", [2 * C, 9, C], bf16, isOutput=False)
    # packed [c2a | c2b | g1 | b1 | g2 | b2], duplicated on both halves
    cst_ext = nc.declare_dram_parameter("cst", [2 * C, 8], f32, isOutput=False)
    out_ext = nc.declare_dram_parameter("out", [B_LOCAL, C, HW, HW], bf16,
                                        isOutput=True)

    with tile.TileContext(nc) as tc:
        with (
            tc.tile_pool(name="big", bufs=1) as big,
            tc.tile_pool(name="small", bufs=1) as pool,
            tc.tile_pool(name="psum", bufs=8, space="PSUM") as psum,
            tc.tile_pool(name="dram", bufs=1, space="DRAM") as dram,
        ):
            w1 = pool.tile([2 * C, 9, C], bf16, tag="w1")
            w2 = pool.tile([2 * C, 9, C], bf16, tag="w2")
            cst = pool.tile([2 * C, 8], f32, tag="cst")
            g1, b1 = cst[:, 4:5], cst[:, 5:6]
            g2, b2 = cst[:, 6:7], cst[:, 7:8]
            eps = pool.tile([2 * C, 1], f32, tag="eps")
            nc.vector.memset(eps[:, :], BN_EPS)
            # constants via the gpsimd SWDGE ring (SP/ACT rings carry x)
            nc.gpsimd.dma_start(out=w1[:, :, :], in_=w1_ext[:, :, :])
            nc.gpsimd.dma_start(out=cst[:, :], in_=cst_ext[:, :])
            nc.gpsimd.dma_start(out=w2[:, :, :], in_=w2_ext[:, :, :])

            for r in range(reps):
                xt = [big.tile([2 * C, HP, HP], bf16, tag=f"xt{b}",
                               name=f"xt{b}") for b in range(B_LOCAL)]
                yt = [big.tile([2 * C, HP, HP], bf16, tag=f"yt{b}",
                               name=f"yt{b}") for b in range(B_LOCAL)]
                # d + residual, 128-wide: partitions 0:64 = images 0,1;
                # 64:128 = images 2,3 (slot = b % 2)
                d = big.tile([2 * C, 2, HW, HW], f32, tag="d")
                xres = big.tile([2 * C, 2, HW, HW], f32, tag="xres")
                dout = big.tile([2 * C, 2, HW, HW], bf16, tag="dout")
                if col_pair:
                    sshape = [2 * C, 2 * NGRP]
                else:
                    sshape = [C, B_LOCAL * NGRP]
                sumd1 = pool.tile(sshape, f32, tag="sumd1")
                sumps1 = pool.tile(sshape, f32, tag="sumps1")
                sumd2 = pool.tile(sshape, f32, tag="sumd2")
                sumps2 = pool.tile(sshape, f32, tag="sumps2")
                layer_fn = _build_layer if col_pair else _build_layer_nopair
                stats_fn = (_stats_allreduce if col_pair
                            else _stats_allreduce_nopair)

                if r == 0:
                    # zero the pad borders once (interior-only writes after
                    # this keep them zero). x tiles on DVE, y tiles on Pool.
                    for tiles, eng in ((xt, nc.vector), (yt, nc.gpsimd)):
                        for t in tiles:
                            eng.memset(t[:, 0:1, :], 0.0)
                            eng.memset(t[:, HP - 1:HP, :], 0.0)
                            eng.memset(t[:, :, 0:1], 0.0)
                            eng.memset(t[:, :, HP - 1:HP], 0.0)

                # ---- x (sync ring) + host-computed x^2 (scalar ring) into
                # padded tiles, 2 chunks each, slot-0 images first ----
                chunks = (((0, 32), (1, 33)), ((32, HW), (33, HW + 1)))
                for b, ci_ in ((0, 0), (2, 0), (1, 0), (3, 0),
                               (0, 1), (2, 1), (1, 1), (3, 1)):
                    rows, pr = chunks[ci_]
                    if True:
                        nc.sync.dma_start(
                            out=xt[b][0:C, pr[0]:pr[1], 1:HW + 1],
                            in_=x_ext[b:b + 1, :, rows[0]:rows[1], :]
                                .transpose([1, 0, 2, 3]))
                        nc.scalar.dma_start(
                            out=xt[b][C:2 * C, pr[0]:pr[1], 1:HW + 1],
                            in_=xsq_ext[b:b + 1, :, rows[0]:rows[1], :]
                                .transpose([1, 0, 2, 3]))

                # ---- layer 1 ----
                ev1 = layer_fn(nc, psum, xt, w1, cst, 1, d, sumd1, sumps1)

                gstats1 = stats_fn(nc, pool, dram, sumd1, sumps1, "l1",
                                   no_collective)
                s1, t1 = _bn_affine(nc, pool, gstats1, cst[:, 0:2], g1, b1,
                                    eps, "l1")

                # ---- glue: y = relu(s1*d + t1) (bf16); y^2 on upper ----
                # 3 chunks; slot-0 glue, then L2 slot 0, then slot-1 glue
                # (interleaving with slot-0 evictions on ACT), then L2 slot 1
                gchunks = (((0, 30), (1, 31)), ((30, 56), (31, 57)))

                def glue(b_list, sq_engines):
                    for rows_d, rows_t in gchunks:
                        for b in b_list:
                            pb, i = _pb(b)
                            nc.scalar.activation(
                                out=yt[b][0:C, rows_t[0]:rows_t[1], 1:HW + 1],
                                in_=d[pb:pb + C, i, rows_d[0]:rows_d[1], :],
                                func=AF.Relu, bias=t1[pb:pb + C, 0:1],
                                scale=s1[pb:pb + C, 0:1])
                            sq_in = yt[b][0:C, rows_t[0]:rows_t[1], 1:HW + 1]
                            sq_out = yt[b][C:2 * C, rows_t[0]:rows_t[1],
                                           1:HW + 1]
                            nc.vector.tensor_tensor(
                                out=sq_out, in0=sq_in, in1=sq_in, op=MULT)

                # ---- layer 2 ----
                if col_pair:
                    glue((0, 2, 1, 3), None)
                    ev2 = _build_layer(nc, psum, yt, w2, cst, 3, d, sumd2,
                                       sumps2)
                    # residual copy of x (f32 twin input; a casting DMA
                    # would shatter into per-element descriptors) during the
                    # L2 window, split sync/SWDGE, away from the stats rings
                    for b in range(B_LOCAL):
                        pb, i = _pb(b)
                        ring = nc.sync if b < 2 else nc.gpsimd
                        xr = ring.dma_start(
                            out=xres[pb:pb + C, i, :, :],
                            in_=xf_ext[b:b + 1, :, :, :]
                                .transpose([1, 0, 2, 3]))
                        tile.add_dep_helper(xr.ins, ev2[0].ins,
                                            reason="defer xres into L2")
                else:
                    for b in range(B_LOCAL):
                        pb, i = _pb(b)
                        xr = nc.gpsimd.dma_start(
                            out=xres[pb:pb + C, i, :, :],
                            in_=xf_ext[b:b + 1, :, :, :]
                                .transpose([1, 0, 2, 3]))
                        tile.add_dep_helper(xr.ins, ev1[2 * b].ins,
                                            reason="defer xres past L1")
                    glue((0, 2), (True, True))
                    glue((1, 3), (True, False))
                    ev2 = layer_fn(nc, psum, yt, w2, cst, 3, d, sumd2, sumps2)
                gstats2 = stats_fn(nc, pool, dram, sumd2, sumps2, "l2",
                                   no_collective)
                s2, t2 = _bn_affine(nc, pool, gstats2, cst[:, 2:4], g2, b2,
                                    eps, "l2")

                # ---- final: out = relu(s2*d + t2 + x), 128-wide, bf16 out;
                # DMA split across rings ----
                rings = [nc.sync, nc.scalar, nc.gpsimd]
                ri = 0
                for i in range(2):
                    for q in range(4):
                        rs = slice(14 * q, 14 * q + 14)
                        nc.vector.scalar_tensor_tensor(
                            out=d[:, i, rs, :], in0=d[:, i, rs, :],
                            scalar=s2[:, 0:1], in1=xres[:, i, rs, :],
                            op0=MULT, op1=ADD)
                        nc.scalar.activation(
                            out=dout[:, i, rs, :], in_=d[:, i, rs, :],
                            func=AF.Relu, bias=t2[:, 0:1], scale=1.0)
                        for half in range(2):
                            b = 2 * half + i
                            rings[ri % len(rings)].dma_start(
                                out=out_ext[b:b + 1, :, rs, :].transpose(
                                    [1, 0, 2, 3]),
                                in_=dout[64 * half:64 * half + C, i, rs, :])
                            ri += 1
    nc.compile()
    return nc


_NC_CACHE = None


def _get_nc():
    global _NC_CACHE
    if _NC_CACHE is None:
        _NC_CACHE = build()
    return _NC_CACHE


def _make_in_maps(x, centers1, gamma1, beta1, centers2, gamma2, beta2):
    from ml_dtypes import bfloat16

    def prep_w(centers):
        w = np.empty((2 * C, 9, C), np.float32)
        # centers: [o, d] with d = c*9 + k  ->  w[c, k, o] = -2*centers[o, 9c+k]
        w[:C] = -2.0 * np.ascontiguousarray(
            centers.reshape(C, C, 9).transpose(1, 2, 0))
        w[C:] = 1.0
        return w.astype(bfloat16)

    c1 = np.asarray(centers1, np.float32)
    c2 = np.asarray(centers2, np.float32)
    zero = np.zeros((C,), np.float32)
    # [zero, c2a, zero, c2b, g1, b1, g2, b2] so BN affine can fuse
    # [mu, E[d^2]] into one scalar_tensor_tensor against [zero, c2]
    cst = np.stack([
        zero, (c1 ** 2).sum(1), zero, (c2 ** 2).sum(1),
        np.asarray(gamma1, np.float32), np.asarray(beta1, np.float32),
        np.asarray(gamma2, np.float32), np.asarray(beta2, np.float32),
    ], axis=1).astype(np.float32)
    cst = np.ascontiguousarray(np.tile(cst, (2, 1)))   # duplicate both halves
    common = {
        "w1": prep_w(c1),
        "w2": prep_w(c2),
        "cst": cst,
    }
    xf = np.ascontiguousarray(np.asarray(x, np.float32))
    x = xf.astype(bfloat16)
    xsq = (xf * xf).astype(bfloat16)
    in_maps = []
    for c in range(N_CORES):
        m = dict(common)
        sl = slice(c * B_LOCAL, (c + 1) * B_LOCAL)
        m["x"] = np.ascontiguousarray(x[sl])
        m["xf"] = np.ascontiguousarray(xf[sl])
        m["xsq"] = np.ascontiguousarray(xsq[sl])
        in_maps.append(m)
    return in_maps


def _run(inputs, trace=False, **kw):
    nc = _get_nc()
    in_maps = _make_in_maps(**inputs)
    res = run_bass_kernel_spmd(nc, in_maps, core_ids=list(range(N_CORES)),
                               trace=trace, **kw)
    out = np.concatenate([res.results[c]["out"] for c in range(N_CORES)], axis=0)
    return out.astype(np.float32), res


def kernel(**inputs):
    out, _ = _run(inputs)
    return out


# revision 20
# speedup vs baseline: 1.0757x; 1.0757x over previous
"""Trainium2 Bass kernel for nn_BasicBlock (distance-transform conv BasicBlock).

Computes: relu(bn2(dt_conv2(relu(bn1(dt_conv1(x))))) + x)
where dt_conv is a 3x3 "distance transform conv":
    d[b,o,h,w] = sqrt(||p - c_o||^2),  p = 3x3 zero-padded patch (dim 576)

Strategy (8 NeuronCores, data-parallel over batch 32 -> 4 images/core):
- ||p||^2 - 2 p.c in ONE matmul accumulation group per pixel tile: SBUF
  partitions 0:64 hold x (weights = -2*centers), partitions 64:128 hold x^2
  (weights = 1.0). 9 shifted matmuls (3x3 offsets) accumulate in PSUM,
  K=128, M=64 out channels, N=448 (8 rows x 56).
- PE column-pairing: images (0,2) and (1,3) share one PSUM bank — image A
  accumulates into psum[0:64] (tile_position (0,0)), image B into
  psum[64:128] ((0,64)). The two 64-col PE groups run concurrently,
  ~doubling effective matmul throughput vs M=64 alone.
- All matmul operands bf16 (tolerance 2e-2; d and BN stats stay f32 where
  conditioning demands). Halves SBUF streams + input DMA.
- Evictions 128-wide: d = sqrt(psum + ||c||^2) on ScalarE per image-PAIR,
  accum_out gives per-channel sum(d). sum(d^2) = sum(psum) + n*c2 via one
  128-wide DVE reduce per pair.
- Sync-BN: fold upper/lower halves, [64,2] AllGather across 8 cores +
  local rank-sum, x2 layers.
- Output written bf16 and converted to f32 on host; out-DMA split across
  rings to shrink the post-barrier tail.

kernel(**inputs) takes FULL unsharded inputs, returns FULL output.
Self-contained: shapes/sharding hardcoded; no file reads.
"""
import numpy as np

from concourse import bacc, mybir, tile
from concourse.bass_utils import run_bass_kernel_spmd

f32 = mybir.dt.float32
bf16 = mybir.dt.bfloat16
ADD = mybir.AluOpType.add
MULT = mybir.AluOpType.mult
SUB = mybir.AluOpType.subtract
AF = mybir.ActivationFunctionType

N_CORES = 8
B_LOCAL = 4            # images per core (32 / 8)
C = 64                 # channels (in == out)
HW = 56                # spatial
HP = HW + 2            # padded
RPG = 8                # rows per matmul group (N = 8*56 = 448)
NGRP = 7               # row-groups per image-pair slot (56 / 8)
N_GLOBAL = 32 * HW * HW
BN_EPS = 1e-5


def _pb(b):
    """Partition base and pair-slot index for the 128-wide d/xres layout."""
    return 64 * (b // 2), b % 2


def _build_layer(nc, psum, src, w, cst, ci, d, sumd, sumps, slots=(0, 1)):
    """One dt_conv layer (or one pair-slot of it). src[b] is a [128, HP, HP]
    bf16 tile (x | x^2). Image pair (i, i+2) shares a PSUM bank: i ->
    psum[0:64] (PE cols 0:64), i+2 -> psum[64:128] (PE cols 64:128), running
    concurrently. d is [128, 2, HW, HW] f32; sumd/sumps [128, 2*NGRP].
    slots selects which image pairs to emit — callers phase slot 1 after
    its glue so ACT evictions interleave with glue instead of queueing
    behind it (ACT is strict FIFO; stuck evictions stall PSUM reuse)."""
    evicts = []
    order = [(g, i) for g in range(3) for i in slots] + \
            [(g, i) for g in range(3, NGRP) for i in slots]
    for g, i in order:
        ps = psum.tile([2 * C, RPG, 64], f32, tag="ps")
        r0 = g * RPG
        for k in range(9):
            kh, kw = k // 3, k % 3
            nc.tensor.matmul(
                ps[0:C, 0:RPG, 0:HW],
                w[:, k, :],
                src[i][:, r0 + kh:r0 + kh + RPG, kw:kw + HW],
                start=(k == 0), stop=(k == 8),
            )
            nc.tensor.matmul(
                ps[C:2 * C, 0:RPG, 0:HW],
                w[:, k, :],
                src[i + 2][:, r0 + kh:r0 + kh + RPG, kw:kw + HW],
                start=(k == 0), stop=(k == 8),
            )
        col = i * NGRP + g
        # per-channel sum(psum) (-> sum(d^2) after +n*c2); emitted first so
        # it runs concurrently with the ACT eviction
        nc.vector.tensor_reduce(
            out=sumps[:, col:col + 1],
            in_=ps[:, 0:RPG, 0:HW],
            axis=mybir.AxisListType.XY, op=ADD)
        # d = sqrt(psum + ||c||^2); accum_out gives per-channel sum(d)
        ev = nc.scalar.activation(
            out=d[:, i, r0:r0 + RPG, :],
            in_=ps[:, 0:RPG, 0:HW],
            func=AF.Sqrt, bias=cst[:, ci:ci + 1], scale=1.0,
            accum_out=sumd[:, col:col + 1])
        evicts.append(ev)
    return evicts


def _build_layer_nopair(nc, psum, src, w, cst, ci, d, sumd, sumps):
    """A/B variant: no PSUM column pairing — per-image groups, M=64 at
    tile_position (0,0), 64-wide evictions (baseline-style)."""
    evicts = []
    order = [(g, b) for g in range(3) for b in range(B_LOCAL)] + \
            [(g, b) for g in range(3, NGRP) for b in range(B_LOCAL)]
    for g, b in order:
        pb, i = _pb(b)
        ps = psum.tile([C, RPG, 64], f32, tag="psn")
        r0 = g * RPG
        for k in range(9):
            kh, kw = k // 3, k % 3
            nc.tensor.matmul(
                ps[:, 0:RPG, 0:HW],
                w[:, k, :],
                src[b][:, r0 + kh:r0 + kh + RPG, kw:kw + HW],
                start=(k == 0), stop=(k == 8),
            )
        col = b * NGRP + g
        nc.vector.tensor_reduce(
            out=sumps[:, col:col + 1],
            in_=ps[:, 0:RPG, 0:HW],
            axis=mybir.AxisListType.XY, op=ADD)
        ev = nc.scalar.activation(
            out=d[pb:pb + C, i, r0:r0 + RPG, :],
            in_=ps[:, 0:RPG, 0:HW],
            func=AF.Sqrt, bias=cst[pb:pb + C, ci:ci + 1], scale=1.0,
            accum_out=sumd[:, col:col + 1])
        evicts.append(ev)
    return evicts


def _stats_allreduce_nopair(nc, pool, dram, sumd, sumps, name,
                            no_collective=False):
    red = pool.tile([C, 2], f32, tag=f"redn_{name}")
    gstats = pool.tile([2 * C, 2], f32, tag=f"gstatsn_{name}")
    nc.vector.tensor_reduce(out=red[:, 0:1], in_=sumd[:, :],
                            axis=mybir.AxisListType.X, op=ADD)
    nc.vector.tensor_reduce(out=red[:, 1:2], in_=sumps[:, :],
                            axis=mybir.AxisListType.X, op=ADD)
    if no_collective:
        nc.vector.tensor_copy(out=gstats[0:C, :], in_=red[:, :])
        nc.vector.tensor_copy(out=gstats[C:2 * C, :], in_=gstats[0:C, :])
        return gstats
    cc_in = dram.tile([C, 2], f32, tag=f"ccinn_{name}")
    cc_out = dram.tile([N_CORES * C, 2], f32, tag=f"ccoutn_{name}")
    gag = pool.tile([C, N_CORES, 2], f32, tag=f"gagn_{name}")
    nc.sync.dma_start(out=cc_in[:, :], in_=red[:, :])
    nc.gpsimd.collective_compute(
        "AllGather", mybir.AluOpType.bypass,
        replica_groups=[list(range(N_CORES))],
        ins=[cc_in.opt()],
        outs=[cc_out.opt()],
    )
    nc.sync.dma_start(
        out=gag[:, :, :],
        in_=cc_out[:, :].rearrange("(r c) s -> c r s", r=N_CORES))
    nc.vector.tensor_reduce(out=gstats[0:C, 0:1], in_=gag[:, :, 0],
                            axis=mybir.AxisListType.X, op=ADD)
    nc.vector.tensor_reduce(out=gstats[0:C, 1:2], in_=gag[:, :, 1],
                            axis=mybir.AxisListType.X, op=ADD)
    nc.vector.tensor_copy(out=gstats[C:2 * C, :], in_=gstats[0:C, :])
    return gstats


def _bn_affine(nc, pool, gstats, zc2, gamma, beta, eps, name):
    """From [sum(d), sum(psum)] (dup both halves) -> scale s, shift t [128,1].
    zc2 is a [128, 2] cst slice [zero, c2]."""
    P = 2 * C
    mued = pool.tile([P, 2], f32, tag=f"mued_{name}")
    mu2 = pool.tile([P, 1], f32, tag=f"mu2_{name}")
    var = pool.tile([P, 1], f32, tag=f"var_{name}")
    sd = pool.tile([P, 1], f32, tag=f"sd_{name}")
    inv = pool.tile([P, 1], f32, tag=f"inv_{name}")
    s = pool.tile([P, 1], f32, tag=f"s_{name}")
    st = pool.tile([P, 1], f32, tag=f"st_{name}")
    tt = pool.tile([P, 1], f32, tag=f"t_{name}")
    inv_n = 1.0 / float(N_GLOBAL)
    # [mu, E[d^2]] = gstats * 1/N + [0, c2] in one DVE op
    nc.vector.scalar_tensor_tensor(
        out=mued[:, :], in0=gstats[:, 0:2], scalar=inv_n, in1=zc2,
        op0=MULT, op1=ADD)
    mu, ed2 = mued[:, 0:1], mued[:, 1:2]
    nc.vector.tensor_tensor(out=mu2[:, :], in0=mu, in1=mu, op=MULT)
    nc.vector.tensor_tensor(out=var[:, :], in0=ed2, in1=mu2[:, :], op=SUB)
    nc.scalar.activation(out=sd[:, :], in_=var[:, :], func=AF.Sqrt,
                         bias=eps[:, 0:1], scale=1.0)
    nc.vector.reciprocal(out=inv[:, :], in_=sd[:, :])
    nc.vector.tensor_tensor(out=s[:, :], in0=gamma, in1=inv[:, :], op=MULT)
    nc.vector.tensor_tensor(out=st[:, :], in0=mu, in1=s[:, :], op=MULT)
    nc.vector.tensor_tensor(out=tt[:, :], in0=beta, in1=st[:, :], op=SUB)
    return s, tt


def _stats_allreduce(nc, pool, dram, sumd, sumps, name, no_collective=False):
    """Reduce [128, 2*NGRP] stat columns, fold upper half into lower,
    AllGather [64,2] across 8 cores + local rank-sum, return [128,2]
    duplicated global sums."""
    red = pool.tile([2 * C, 2], f32, tag=f"red_{name}")
    gstats = pool.tile([2 * C, 2], f32, tag=f"gstats_{name}")
    nc.vector.tensor_reduce(out=red[:, 0:1], in_=sumd[:, :],
                            axis=mybir.AxisListType.X, op=ADD)
    nc.vector.tensor_reduce(out=red[:, 1:2], in_=sumps[:, :],
                            axis=mybir.AxisListType.X, op=ADD)
    # images (2,3) stats live on the upper partition half; DMA both halves
    # side by side into the collective input (DVE tensor ops can't mix
    # partition bases, DMA can) and fold during the rank-sum reduce.
    cc_in = dram.tile([C, 2, 2], f32, tag=f"ccin_{name}")
    # one DMA: partition-major red [128,2] -> (h, c, s) walk of cc_in
    nc.sync.dma_start(out=cc_in[:, :, :].rearrange("c s h -> h c s"),
                      in_=red[:, :])
    if no_collective:
        gag = pool.tile([C, 2, 2], f32, tag=f"gag_{name}")
        nc.sync.dma_start(out=gag[:, :, :], in_=cc_in[:, :, :])
        nc.vector.tensor_reduce(out=gstats[0:C, 0:1], in_=gag[:, 0, :],
                                axis=mybir.AxisListType.X, op=ADD)
        nc.vector.tensor_reduce(out=gstats[0:C, 1:2], in_=gag[:, 1, :],
                                axis=mybir.AxisListType.X, op=ADD)
        nc.vector.tensor_copy(out=gstats[C:2 * C, :], in_=gstats[0:C, :])
        return gstats
    # AllGather (floor ~4.6us vs AllReduce ~9.7us) + local rank-sum.
    cc_out = dram.tile([N_CORES * C, 2, 2], f32, tag=f"ccout_{name}")
    gag = pool.tile([C, N_CORES, 2, 2], f32, tag=f"gag_{name}")
    nc.gpsimd.collective_compute(
        "AllGather", mybir.AluOpType.bypass,
        replica_groups=[list(range(N_CORES))],
        ins=[cc_in.opt()],
        outs=[cc_out.opt()],
    )
    nc.sync.dma_start(
        out=gag[:, :, :, :],
        in_=cc_out[:, :, :].rearrange("(r c) s h -> c r s h", r=N_CORES))
    nc.vector.tensor_reduce(out=gstats[0:C, 0:1], in_=gag[:, :, 0, :],
                            axis=mybir.AxisListType.XY, op=ADD)
    nc.vector.tensor_reduce(out=gstats[0:C, 1:2], in_=gag[:, :, 1, :],
                            axis=mybir.AxisListType.XY, op=ADD)
    nc.vector.tensor_copy(out=gstats[C:2 * C, :], in_=gstats[0:C, :])
    return gstats


def build(no_collective=False, reps=1, col_pair=True):
    nc = bacc.Bacc("TRN2", target_bir_lowering=False, debug=False,
                   num_devices=1 if no_collective else N_CORES)
    x_ext = nc.declare_dram_parameter("x", [B_LOCAL, C, HW, HW], bf16,
                                      isOutput=False)
    xf_ext = nc.declare_dram_parameter("xf", [B_LOCAL, C, HW, HW], f32,
                                       isOutput=False)
    xsq_ext = nc.declare_dram_parameter("xsq", [B_LOCAL, C, HW, HW], bf16,
                                        isOutput=False)
    w1_ext = nc.declare_dram_parameter("w1", [2 * C, 9, C], bf16, isOutput=False)
    w2_ext = nc.declare_dram_parameter("w2", [2 * C, 9, C], bf16, isOutput=False)
    # packed [c2a | c2b | g1 | b1 | g2 | b2], duplicated on both halves
    cst_ext = nc.declare_dram_parameter("cst", [2 * C, 8], f32, isOutput=False)
    out_ext = nc.declare_dram_parameter("out", [B_LOCAL, C, HW, HW], bf16,
                                        isOutput=True)

    with tile.TileContext(nc) as tc:
        with (
            tc.tile_pool(name="big", bufs=1) as big,
            tc.tile_pool(name="small", bufs=1) as pool,
            tc.tile_pool(name="psum", bufs=8, space="PSUM") as psum,
            tc.tile_pool(name="dram", bufs=1, space="DRAM") as dram,
        ):
            w1 = pool.tile([2 * C, 9, C], bf16, tag="w1")
            w2 = pool.tile([2 * C, 9, C], bf16, tag="w2")
            cst = pool.tile([2 * C, 8], f32, tag="cst")
            g1, b1 = cst[:, 4:5], cst[:, 5:6]
            g2, b2 = cst[:, 6:7], cst[:, 7:8]
            eps = pool.tile([2 * C, 1], f32, tag="eps")
            nc.vector.memset(eps[:, :], BN_EPS)
            # constants via the gpsimd SWDGE ring (SP/ACT rings carry x)
            nc.gpsimd.dma_start(out=w1[:, :, :], in_=w1_ext[:, :, :])
            nc.gpsimd.dma_start(out=cst[:, :], in_=cst_ext[:, :])
            nc.gpsimd.dma_start(out=w2[:, :, :], in_=w2_ext[:, :, :])

            for r in range(reps):
                xt = [big.tile([2 * C, HP, HP], bf16, tag=f"xt{b}",
                               name=f"xt{b}") for b in range(B_LOCAL)]
                yt = [big.tile([2 * C, HP, HP], bf16, tag=f"yt{b}",
                               name=f"yt{b}") for b in range(B_LOCAL)]
                # d + residual, 128-wide: partitions 0:64 = images 0,1;
                # 64:128 = images 2,3 (slot = b % 2)
                d = big.tile([2 * C, 2, HW, HW], f32, tag="d")
                xres = big.tile([2 * C, 2, HW, HW], f32, tag="xres")
                dout = big.tile([2 * C, 2, HW, HW], bf16, tag="dout")
                if col_pair:
                    sshape = [2 * C, 2 * NGRP]
                else:
                    sshape = [C, B_LOCAL * NGRP]
                sumd1 = pool.tile(sshape, f32, tag="sumd1")
                sumps1 = pool.tile(sshape, f32, tag="sumps1")
                sumd2 = pool.tile(sshape, f32, tag="sumd2")
                sumps2 = pool.tile(sshape, f32, tag="sumps2")
                layer_fn = _build_layer if col_pair else _build_layer_nopair
                stats_fn = (_stats_allreduce if col_pair
                            else _stats_allreduce_nopair)

                if r == 0:
                    # zero the pad borders once (interior-only writes after
                    # this keep them zero). x tiles on DVE, y tiles on Pool.
                    for tiles, eng in ((xt, nc.vector), (yt, nc.gpsimd)):
                        for t in tiles:
                            eng.memset(t[:, 0:1, :], 0.0)
                            eng.memset(t[:, HP - 1:HP, :], 0.0)
                            eng.memset(t[:, :, 0:1], 0.0)
                            eng.memset(t[:, :, HP - 1:HP], 0.0)

                # ---- x into padded tiles (2 chunks for startup latency);
                # squares on DVE ----
                dmaeng = {0: nc.sync, 2: nc.scalar, 1: nc.sync, 3: nc.scalar}
                for b in (0, 2, 1, 3):
                    dmaeng[b].dma_start(
                        out=xt[b][0:C, 1:33, 1:HW + 1],
                        in_=x_ext[b:b + 1, :, 0:32, :].transpose([1, 0, 2, 3]))
                    nc.vector.tensor_tensor(
                        out=xt[b][C:2 * C, 1:33, 1:HW + 1],
                        in0=xt[b][0:C, 1:33, 1:HW + 1],
                        in1=xt[b][0:C, 1:33, 1:HW + 1], op=MULT)
                for b in (0, 2, 1, 3):
                    dmaeng[b].dma_start(
                        out=xt[b][0:C, 33:HW + 1, 1:HW + 1],
                        in_=x_ext[b:b + 1, :, 32:HW, :].transpose([1, 0, 2, 3]))
                    nc.vector.tensor_tensor(
                        out=xt[b][C:2 * C, 33:HW + 1, 1:HW + 1],
                        in0=xt[b][0:C, 33:HW + 1, 1:HW + 1],
                        in1=xt[b][0:C, 33:HW + 1, 1:HW + 1], op=MULT)

                # ---- layer 1 ----
                ev1 = layer_fn(nc, psum, xt, w1, cst, 1, d, sumd1, sumps1)

                # residual copy of x (f32 twin input; a casting DMA would
                # shatter into per-element descriptors), 128-wide layout;
                # needed only at the end, so defer past L1 start
                for b in range(B_LOCAL):
                    pb, i = _pb(b)
                    xr = nc.gpsimd.dma_start(
                        out=xres[pb:pb + C, i, :, :],
                        in_=xf_ext[b:b + 1, :, :, :].transpose([1, 0, 2, 3]))
                    tile.add_dep_helper(xr.ins, ev1[2 * b].ins,
                                        reason="defer xres DMA past L1 start")
                gstats1 = stats_fn(nc, pool, dram, sumd1, sumps1, "l1",
                                   no_collective)
                s1, t1 = _bn_affine(nc, pool, gstats1, cst[:, 0:2], g1, b1,
                                    eps, "l1")

                # ---- glue: y = relu(s1*d + t1) (bf16); y^2 on upper ----
                # 3 chunks; slot-0 glue, then L2 slot 0, then slot-1 glue
                # (interleaving with slot-0 evictions on ACT), then L2 slot 1
                gchunks = (((0, 30), (1, 31)), ((30, 56), (31, 57)))

                def glue(b_list, sq_engines):
                    for rows_d, rows_t in gchunks:
                        for b in b_list:
                            pb, i = _pb(b)
                            nc.scalar.activation(
                                out=yt[b][0:C, rows_t[0]:rows_t[1], 1:HW + 1],
                                in_=d[pb:pb + C, i, rows_d[0]:rows_d[1], :],
                                func=AF.Relu, bias=t1[pb:pb + C, 0:1],
                                scale=s1[pb:pb + C, 0:1])
                            sq_in = yt[b][0:C, rows_t[0]:rows_t[1], 1:HW + 1]
                            sq_out = yt[b][C:2 * C, rows_t[0]:rows_t[1],
                                           1:HW + 1]
                            nc.vector.tensor_tensor(
                                out=sq_out, in0=sq_in, in1=sq_in, op=MULT)

                # ---- layer 2 ----
                if col_pair:
                    glue((0, 2, 1, 3), None)
                    ev2 = _build_layer(nc, psum, yt, w2, cst, 3, d, sumd2,
                                       sumps2)
                else:
                    for b in range(B_LOCAL):
                        pb, i = _pb(b)
                        xr = nc.gpsimd.dma_start(
                            out=xres[pb:pb + C, i, :, :],
                            in_=xf_ext[b:b + 1, :, :, :]
                                .transpose([1, 0, 2, 3]))
                        tile.add_dep_helper(xr.ins, ev1[2 * b].ins,
                                            reason="defer xres past L1")
                    glue((0, 2), (True, True))
                    glue((1, 3), (True, False))
                    ev2 = layer_fn(nc, psum, yt, w2, cst, 3, d, sumd2, sumps2)
                gstats2 = stats_fn(nc, pool, dram, sumd2, sumps2, "l2",
                                   no_collective)
                s2, t2 = _bn_affine(nc, pool, gstats2, cst[:, 2:4], g2, b2,
                                    eps, "l2")

                # ---- final: out = relu(s2*d + t2 + x), 128-wide, bf16 out;
                # DMA split across rings ----
                rings = [nc.sync, nc.gpsimd, nc.sync, nc.gpsimd,
                         nc.sync, nc.gpsimd, nc.scalar, nc.scalar]
                ri = 0
                for i in range(2):
                    for q in range(4):
                        rs = slice(14 * q, 14 * q + 14)
                        nc.vector.scalar_tensor_tensor(
                            out=d[:, i, rs, :], in0=d[:, i, rs, :],
                            scalar=s2[:, 0:1], in1=xres[:, i, rs, :],
                            op0=MULT, op1=ADD)
                        nc.scalar.activation(
                            out=dout[:, i, rs, :], in_=d[:, i, rs, :],
                            func=AF.Relu, bias=t2[:, 0:1], scale=1.0)
                        for half in range(2):
                            b = 2 * half + i
                            rings[ri % len(rings)].dma_start(
                                out=out_ext[b:b + 1, :, rs, :].transpose(
                                    [1, 0, 2, 3]),
                                in_=dout[64 * half:64 * half + C, i, rs, :])
                            ri += 1
    nc.compile()
    return nc


_NC_CACHE = None


def _get_nc():
    global _NC_CACHE
    if _NC_CACHE is None:
        _NC_CACHE = build()
    return _NC_CACHE


def _make_in_maps(x, centers1, gamma1, beta1, centers2, gamma2, beta2):
    from ml_dtypes import bfloat16

    def prep_w(centers):
        w = np.empty((2 * C, 9, C), np.float32)
        # centers: [o, d] with d = c*9 + k  ->  w[c, k, o] = -2*centers[o, 9c+k]
        w[:C] = -2.0 * np.ascontiguousarray(
            centers.reshape(C, C, 9).transpose(1, 2, 0))
        w[C:] = 1.0
        return w.astype(bfloat16)

    c1 = np.asarray(centers1, np.float32)
    c2 = np.asarray(centers2, np.float32)
    zero = np.zeros((C,), np.float32)
    # [zero, c2a, zero, c2b, g1, b1, g2, b2] so BN affine can fuse
    # [mu, E[d^2]] into one scalar_tensor_tensor against [zero, c2]
    cst = np.stack([
        zero, (c1 ** 2).sum(1), zero, (c2 ** 2).sum(1),
        np.asarray(gamma1, np.float32), np.asarray(beta1, np.float32),
        np.asarray(gamma2, np.float32), np.asarray(beta2, np.float32),
    ], axis=1).astype(np.float32)
    cst = np.ascontiguousarray(np.tile(cst, (2, 1)))   # duplicate both halves
    common = {
        "w1": prep_w(c1),
        "w2": prep_w(c2),
        "cst": cst,
    }
    xf = np.ascontiguousarray(np.asarray(x, np.float32))
    x = xf.astype(bfloat16)
    xsq = (xf * xf).astype(bfloat16)
    in_maps = []
    for c in range(N_CORES):
        m = dict(common)
        sl = slice(c * B_LOCAL, (c + 1) * B_LOCAL)
        m["x"] = np.ascontiguousarray(x[sl])
        m["xf"] = np.ascontiguousarray(xf[sl])
        m["xsq"] = np.ascontiguousarray(xsq[sl])
        in_maps.append(m)
    return in_maps


def _run(inputs, trace=False, **kw):
    nc = _get_nc()
    in_maps = _make_in_maps(**inputs)
    res = run_bass_kernel_spmd(nc, in_maps, core_ids=list(range(N_CORES)),
                               trace=trace, **kw)
    out = np.concatenate([res.results[c]["out"] for c in range(N_CORES)], axis=0)
    return out.astype(np.float32), res


def kernel(**inputs):
    out, _ = _run(inputs)
    return out


# revision 21
# speedup vs baseline: 1.0972x; 1.0200x over previous
"""Trainium2 Bass kernel for nn_BasicBlock (distance-transform conv BasicBlock).

Computes: relu(bn2(dt_conv2(relu(bn1(dt_conv1(x))))) + x)
where dt_conv is a 3x3 "distance transform conv":
    d[b,o,h,w] = sqrt(||p - c_o||^2),  p = 3x3 zero-padded patch (dim 576)

Strategy (8 NeuronCores, data-parallel over batch 32 -> 4 images/core):
- ||p||^2 - 2 p.c in ONE matmul accumulation group per pixel tile: SBUF
  partitions 0:64 hold x (weights = -2*centers), partitions 64:128 hold x^2
  (weights = 1.0). 9 shifted matmuls (3x3 offsets) accumulate in PSUM,
  K=128, M=64 out channels, N=448 (8 rows x 56).
- PE column-pairing: images (0,2) and (1,3) share one PSUM bank — image A
  accumulates into psum[0:64] (tile_position (0,0)), image B into
  psum[64:128] ((0,64)). The two 64-col PE groups run concurrently,
  ~doubling effective matmul throughput vs M=64 alone.
- All matmul operands bf16 (tolerance 2e-2; d and BN stats stay f32 where
  conditioning demands). Halves SBUF streams + input DMA.
- Evictions 128-wide: d = sqrt(psum + ||c||^2) on ScalarE per image-PAIR,
  accum_out gives per-channel sum(d). sum(d^2) = sum(psum) + n*c2 via one
  128-wide DVE reduce per pair.
- Sync-BN: fold upper/lower halves, [64,2] AllGather across 8 cores +
  local rank-sum, x2 layers.
- Output written bf16 and converted to f32 on host; out-DMA split across
  rings to shrink the post-barrier tail.

kernel(**inputs) takes FULL unsharded inputs, returns FULL output.
Self-contained: shapes/sharding hardcoded; no file reads.
"""
import numpy as np

from concourse import bacc, mybir, tile
from concourse.bass_utils import run_bass_kernel_spmd

f32 = mybir.dt.float32
bf16 = mybir.dt.bfloat16
ADD = mybir.AluOpType.add
MULT = mybir.AluOpType.mult
SUB = mybir.AluOpType.subtract
AF = mybir.ActivationFunctionType

N_CORES = 8
B_LOCAL = 4            # images per core (32 / 8)
C = 64                 # channels (in == out)
HW = 56                # spatial
HP = HW + 2            # padded
RPG = 8                # rows per matmul group (N = 8*56 = 448)
NGRP = 7               # row-groups per image-pair slot (56 / 8)
N_GLOBAL = 32 * HW * HW
BN_EPS = 1e-5


def _pb(b):
    """Partition base and pair-slot index for the 128-wide d/xres layout."""
    return 64 * (b // 2), b % 2


def _build_layer(nc, psum, src, w, cst, ci, d, sumd, sumps, slots=(0, 1)):
    """One dt_conv layer (or one pair-slot of it). src[b] is a [128, HP, HP]
    bf16 tile (x | x^2). Image pair (i, i+2) shares a PSUM bank: i ->
    psum[0:64] (PE cols 0:64), i+2 -> psum[64:128] (PE cols 64:128), running
    concurrently. d is [128, 2, HW, HW] f32; sumd/sumps [128, 2*NGRP].
    slots selects which image pairs to emit — callers phase slot 1 after
    its glue so ACT evictions interleave with glue instead of queueing
    behind it (ACT is strict FIFO; stuck evictions stall PSUM reuse)."""
    evicts = []
    order = [(g, i) for g in range(3) for i in slots] + \
            [(g, i) for g in range(3, NGRP) for i in slots]
    for g, i in order:
        ps = psum.tile([2 * C, RPG, 64], f32, tag="ps")
        r0 = g * RPG
        for k in range(9):
            kh, kw = k // 3, k % 3
            nc.tensor.matmul(
                ps[0:C, 0:RPG, 0:HW],
                w[:, k, :],
                src[i][:, r0 + kh:r0 + kh + RPG, kw:kw + HW],
                start=(k == 0), stop=(k == 8),
            )
            nc.tensor.matmul(
                ps[C:2 * C, 0:RPG, 0:HW],
                w[:, k, :],
                src[i + 2][:, r0 + kh:r0 + kh + RPG, kw:kw + HW],
                start=(k == 0), stop=(k == 8),
            )
        col = i * NGRP + g
        # per-channel sum(psum) (-> sum(d^2) after +n*c2); emitted first so
        # it runs concurrently with the ACT eviction
        nc.vector.tensor_reduce(
            out=sumps[:, col:col + 1],
            in_=ps[:, 0:RPG, 0:HW],
            axis=mybir.AxisListType.XY, op=ADD)
        # d = sqrt(psum + ||c||^2); accum_out gives per-channel sum(d)
        ev = nc.scalar.activation(
            out=d[:, i, r0:r0 + RPG, :],
            in_=ps[:, 0:RPG, 0:HW],
            func=AF.Sqrt, bias=cst[:, ci:ci + 1], scale=1.0,
            accum_out=sumd[:, col:col + 1])
        evicts.append(ev)
    return evicts


def _build_layer_nopair(nc, psum, src, w, cst, ci, d, sumd, sumps):
    """A/B variant: no PSUM column pairing — per-image groups, M=64 at
    tile_position (0,0), 64-wide evictions (baseline-style)."""
    evicts = []
    order = [(g, b) for g in range(3) for b in range(B_LOCAL)] + \
            [(g, b) for g in range(3, NGRP) for b in range(B_LOCAL)]
    for g, b in order:
        pb, i = _pb(b)
        ps = psum.tile([C, RPG, 64], f32, tag="psn")
        r0 = g * RPG
        for k in range(9):
            kh, kw = k // 3, k % 3
            nc.tensor.matmul(
                ps[:, 0:RPG, 0:HW],
                w[:, k, :],
                src[b][:, r0 + kh:r0 + kh + RPG, kw:kw + HW],
                start=(k == 0), stop=(k == 8),
            )
        col = b * NGRP + g
        nc.vector.tensor_reduce(
            out=sumps[:, col:col + 1],
            in_=ps[:, 0:RPG, 0:HW],
            axis=mybir.AxisListType.XY, op=ADD)
        ev = nc.scalar.activation(
            out=d[pb:pb + C, i, r0:r0 + RPG, :],
            in_=ps[:, 0:RPG, 0:HW],
            func=AF.Sqrt, bias=cst[pb:pb + C, ci:ci + 1], scale=1.0,
            accum_out=sumd[:, col:col + 1])
        evicts.append(ev)
    return evicts


def _stats_allreduce_nopair(nc, pool, dram, sumd, sumps, name,
                            no_collective=False):
    red = pool.tile([C, 2], f32, tag=f"redn_{name}")
    gstats = pool.tile([2 * C, 2], f32, tag=f"gstatsn_{name}")
    nc.vector.tensor_reduce(out=red[:, 0:1], in_=sumd[:, :],
                            axis=mybir.AxisListType.X, op=ADD)
    nc.vector.tensor_reduce(out=red[:, 1:2], in_=sumps[:, :],
                            axis=mybir.AxisListType.X, op=ADD)
    if no_collective:
        nc.vector.tensor_copy(out=gstats[0:C, :], in_=red[:, :])
        nc.vector.tensor_copy(out=gstats[C:2 * C, :], in_=gstats[0:C, :])
        return gstats
    cc_in = dram.tile([C, 2], f32, tag=f"ccinn_{name}")
    cc_out = dram.tile([N_CORES * C, 2], f32, tag=f"ccoutn_{name}")
    gag = pool.tile([C, N_CORES, 2], f32, tag=f"gagn_{name}")
    nc.sync.dma_start(out=cc_in[:, :], in_=red[:, :])
    nc.gpsimd.collective_compute(
        "AllGather", mybir.AluOpType.bypass,
        replica_groups=[list(range(N_CORES))],
        ins=[cc_in.opt()],
        outs=[cc_out.opt()],
    )
    nc.sync.dma_start(
        out=gag[:, :, :],
        in_=cc_out[:, :].rearrange("(r c) s -> c r s", r=N_CORES))
    nc.vector.tensor_reduce(out=gstats[0:C, 0:1], in_=gag[:, :, 0],
                            axis=mybir.AxisListType.X, op=ADD)
    nc.vector.tensor_reduce(out=gstats[0:C, 1:2], in_=gag[:, :, 1],
                            axis=mybir.AxisListType.X, op=ADD)
    nc.vector.tensor_copy(out=gstats[C:2 * C, :], in_=gstats[0:C, :])
    return gstats


def _bn_affine(nc, pool, gstats, zc2, gamma, beta, eps, name):
    """From [sum(d), sum(psum)] (dup both halves) -> scale s, shift t [128,1].
    zc2 is a [128, 2] cst slice [zero, c2]."""
    P = 2 * C
    mued = pool.tile([P, 2], f32, tag=f"mued_{name}")
    mu2 = pool.tile([P, 1], f32, tag=f"mu2_{name}")
    var = pool.tile([P, 1], f32, tag=f"var_{name}")
    sd = pool.tile([P, 1], f32, tag=f"sd_{name}")
    inv = pool.tile([P, 1], f32, tag=f"inv_{name}")
    s = pool.tile([P, 1], f32, tag=f"s_{name}")
    st = pool.tile([P, 1], f32, tag=f"st_{name}")
    tt = pool.tile([P, 1], f32, tag=f"t_{name}")
    inv_n = 1.0 / float(N_GLOBAL)
    # [mu, E[d^2]] = gstats * 1/N + [0, c2] in one DVE op
    nc.vector.scalar_tensor_tensor(
        out=mued[:, :], in0=gstats[:, 0:2], scalar=inv_n, in1=zc2,
        op0=MULT, op1=ADD)
    mu, ed2 = mued[:, 0:1], mued[:, 1:2]
    nc.vector.tensor_tensor(out=mu2[:, :], in0=mu, in1=mu, op=MULT)
    nc.vector.tensor_tensor(out=var[:, :], in0=ed2, in1=mu2[:, :], op=SUB)
    nc.scalar.activation(out=sd[:, :], in_=var[:, :], func=AF.Sqrt,
                         bias=eps[:, 0:1], scale=1.0)
    nc.vector.reciprocal(out=inv[:, :], in_=sd[:, :])
    nc.vector.tensor_tensor(out=s[:, :], in0=gamma, in1=inv[:, :], op=MULT)
    nc.vector.tensor_tensor(out=st[:, :], in0=mu, in1=s[:, :], op=MULT)
    nc.vector.tensor_tensor(out=tt[:, :], in0=beta, in1=st[:, :], op=SUB)
    return s, tt


def _stats_allreduce(nc, pool, dram, sumd, sumps, name, no_collective=False):
    """Reduce [128, 2*NGRP] stat columns, fold upper half into lower,
    AllGather [64,2] across 8 cores + local rank-sum, return [128,2]
    duplicated global sums."""
    red = pool.tile([2 * C, 2], f32, tag=f"red_{name}")
    gstats = pool.tile([2 * C, 2], f32, tag=f"gstats_{name}")
    nc.vector.tensor_reduce(out=red[:, 0:1], in_=sumd[:, :],
                            axis=mybir.AxisListType.X, op=ADD)
    nc.vector.tensor_reduce(out=red[:, 1:2], in_=sumps[:, :],
                            axis=mybir.AxisListType.X, op=ADD)
    # images (2,3) stats live on the upper partition half; DMA both halves
    # side by side into the collective input (DVE tensor ops can't mix
    # partition bases, DMA can) and fold during the rank-sum reduce.
    cc_in = dram.tile([C, 2, 2], f32, tag=f"ccin_{name}")
    # one DMA: partition-major red [128,2] -> (h, c, s) walk of cc_in
    nc.sync.dma_start(out=cc_in[:, :, :].rearrange("c s h -> h c s"),
                      in_=red[:, :])
    if no_collective:
        gag = pool.tile([C, 2, 2], f32, tag=f"gag_{name}")
        nc.sync.dma_start(out=gag[:, :, :], in_=cc_in[:, :, :])
        nc.vector.tensor_reduce(out=gstats[0:C, 0:1], in_=gag[:, 0, :],
                                axis=mybir.AxisListType.X, op=ADD)
        nc.vector.tensor_reduce(out=gstats[0:C, 1:2], in_=gag[:, 1, :],
                                axis=mybir.AxisListType.X, op=ADD)
        nc.vector.tensor_copy(out=gstats[C:2 * C, :], in_=gstats[0:C, :])
        return gstats
    # AllGather (floor ~4.6us vs AllReduce ~9.7us) + local rank-sum.
    cc_out = dram.tile([N_CORES * C, 2, 2], f32, tag=f"ccout_{name}")
    gag = pool.tile([C, N_CORES, 2, 2], f32, tag=f"gag_{name}")
    nc.gpsimd.collective_compute(
        "AllGather", mybir.AluOpType.bypass,
        replica_groups=[list(range(N_CORES))],
        ins=[cc_in.opt()],
        outs=[cc_out.opt()],
    )
    nc.sync.dma_start(
        out=gag[:, :, :, :],
        in_=cc_out[:, :, :].rearrange("(r c) s h -> c r s h", r=N_CORES))
    nc.vector.tensor_reduce(out=gstats[0:C, 0:1], in_=gag[:, :, 0, :],
                            axis=mybir.AxisListType.XY, op=ADD)
    nc.vector.tensor_reduce(out=gstats[0:C, 1:2], in_=gag[:, :, 1, :],
                            axis=mybir.AxisListType.XY, op=ADD)
    nc.vector.tensor_copy(out=gstats[C:2 * C, :], in_=gstats[0:C, :])
    return gstats


def build(no_collective=False, reps=1, col_pair=True):
    nc = bacc.Bacc("TRN2", target_bir_lowering=False, debug=False,
                   num_devices=1 if no_collective else N_CORES)
    x_ext = nc.declare_dram_parameter("x", [B_LOCAL, C, HW, HW], bf16,
                                      isOutput=False)
    xf_ext = nc.declare_dram_parameter("xf", [B_LOCAL, C, HW, HW], f32,
                                       isOutput=False)
    xsq_ext = nc.declare_dram_parameter("xsq", [B_LOCAL, C, HW, HW], bf16,
                                        isOutput=False)
    w1_ext = nc.declare_dram_parameter("w1", [2 * C, 9, C], bf16, isOutput=False)
    w2_ext = nc.declare_dram_parameter("w2", [2 * C, 9, C], bf16, isOutput=False)
    # packed [c2a | c2b | g1 | b1 | g2 | b2], duplicated on both halves
    cst_ext = nc.declare_dram_parameter("cst", [2 * C, 8], f32, isOutput=False)
    out_ext = nc.declare_dram_parameter("out", [B_LOCAL, C, HW, HW], bf16,
                                        isOutput=True)

    with tile.TileContext(nc) as tc:
        with (
            tc.tile_pool(name="big", bufs=1) as big,
            tc.tile_pool(name="small", bufs=1) as pool,
            tc.tile_pool(name="psum", bufs=8, space="PSUM") as psum,
            tc.tile_pool(name="dram", bufs=1, space="DRAM") as dram,
        ):
            w1 = pool.tile([2 * C, 9, C], bf16, tag="w1")
            w2 = pool.tile([2 * C, 9, C], bf16, tag="w2")
            cst = pool.tile([2 * C, 8], f32, tag="cst")
            g1, b1 = cst[:, 4:5], cst[:, 5:6]
            g2, b2 = cst[:, 6:7], cst[:, 7:8]
            eps = pool.tile([2 * C, 1], f32, tag="eps")
            nc.vector.memset(eps[:, :], BN_EPS)
            # constants via the gpsimd SWDGE ring (SP/ACT rings carry x)
            nc.gpsimd.dma_start(out=w1[:, :, :], in_=w1_ext[:, :, :])
            nc.gpsimd.dma_start(out=cst[:, :], in_=cst_ext[:, :])
            nc.gpsimd.dma_start(out=w2[:, :, :], in_=w2_ext[:, :, :])

            for r in range(reps):
                xt = [big.tile([2 * C, HP, HP], bf16, tag=f"xt{b}",
                               name=f"xt{b}") for b in range(B_LOCAL)]
                yt = [big.tile([2 * C, HP, HP], bf16, tag=f"yt{b}",
                               name=f"yt{b}") for b in range(B_LOCAL)]
                # d + residual, 128-wide: partitions 0:64 = images 0,1;
                # 64:128 = images 2,3 (slot = b % 2)
                d = big.tile([2 * C, 2, HW, HW], f32, tag="d")
                xres = big.tile([2 * C, 2, HW, HW], f32, tag="xres")
                dout = big.tile([2 * C, 2, HW, HW], bf16, tag="dout")
                if col_pair:
                    sshape = [2 * C, 2 * NGRP]
                else:
                    sshape = [C, B_LOCAL * NGRP]
                sumd1 = pool.tile(sshape, f32, tag="sumd1")
                sumps1 = pool.tile(sshape, f32, tag="sumps1")
                sumd2 = pool.tile(sshape, f32, tag="sumd2")
                sumps2 = pool.tile(sshape, f32, tag="sumps2")
                layer_fn = _build_layer if col_pair else _build_layer_nopair
                stats_fn = (_stats_allreduce if col_pair
                            else _stats_allreduce_nopair)

                if r == 0:
                    # zero the pad borders once (interior-only writes after
                    # this keep them zero). x tiles on DVE, y tiles on Pool.
                    for tiles, eng in ((xt, nc.vector), (yt, nc.gpsimd)):
                        for t in tiles:
                            eng.memset(t[:, 0:1, :], 0.0)
                            eng.memset(t[:, HP - 1:HP, :], 0.0)
                            eng.memset(t[:, :, 0:1], 0.0)
                            eng.memset(t[:, :, HP - 1:HP], 0.0)

                # ---- x (sync ring) + host-computed x^2 (scalar ring) into
                # padded tiles, 2 chunks each ----
                for rows, pr in (((0, 32), (1, 33)), ((32, HW), (33, HW + 1))):
                    for b in (0, 2, 1, 3):
                        nc.sync.dma_start(
                            out=xt[b][0:C, pr[0]:pr[1], 1:HW + 1],
                            in_=x_ext[b:b + 1, :, rows[0]:rows[1], :]
                                .transpose([1, 0, 2, 3]))
                        nc.scalar.dma_start(
                            out=xt[b][C:2 * C, pr[0]:pr[1], 1:HW + 1],
                            in_=xsq_ext[b:b + 1, :, rows[0]:rows[1], :]
                                .transpose([1, 0, 2, 3]))

                # ---- layer 1 ----
                ev1 = layer_fn(nc, psum, xt, w1, cst, 1, d, sumd1, sumps1)

                # residual copy of x (f32 twin input; a casting DMA would
                # shatter into per-element descriptors), 128-wide layout;
                # needed only at the end, so defer past L1 start
                for b in range(B_LOCAL):
                    pb, i = _pb(b)
                    xr = nc.gpsimd.dma_start(
                        out=xres[pb:pb + C, i, :, :],
                        in_=xf_ext[b:b + 1, :, :, :].transpose([1, 0, 2, 3]))
                    tile.add_dep_helper(xr.ins, ev1[2 * b].ins,
                                        reason="defer xres DMA past L1 start")
                gstats1 = stats_fn(nc, pool, dram, sumd1, sumps1, "l1",
                                   no_collective)
                s1, t1 = _bn_affine(nc, pool, gstats1, cst[:, 0:2], g1, b1,
                                    eps, "l1")

                # ---- glue: y = relu(s1*d + t1) (bf16); y^2 on upper ----
                # 3 chunks; slot-0 glue, then L2 slot 0, then slot-1 glue
                # (interleaving with slot-0 evictions on ACT), then L2 slot 1
                gchunks = (((0, 30), (1, 31)), ((30, 56), (31, 57)))

                def glue(b_list, sq_engines):
                    for rows_d, rows_t in gchunks:
                        for b in b_list:
                            pb, i = _pb(b)
                            nc.scalar.activation(
                                out=yt[b][0:C, rows_t[0]:rows_t[1], 1:HW + 1],
                                in_=d[pb:pb + C, i, rows_d[0]:rows_d[1], :],
                                func=AF.Relu, bias=t1[pb:pb + C, 0:1],
                                scale=s1[pb:pb + C, 0:1])
                            sq_in = yt[b][0:C, rows_t[0]:rows_t[1], 1:HW + 1]
                            sq_out = yt[b][C:2 * C, rows_t[0]:rows_t[1],
                                           1:HW + 1]
                            if b in (0, 2):
                                nc.vector.tensor_tensor(
                                    out=sq_out, in0=sq_in, in1=sq_in, op=MULT)
                            else:
                                nc.scalar.activation(
                                    out=sq_out, in_=sq_in, func=AF.Square)

                # ---- layer 2 ----
                if col_pair:
                    glue((0, 2, 1, 3), None)
                    ev2 = _build_layer(nc, psum, yt, w2, cst, 3, d, sumd2,
                                       sumps2)
                else:
                    for b in range(B_LOCAL):
                        pb, i = _pb(b)
                        xr = nc.gpsimd.dma_start(
                            out=xres[pb:pb + C, i, :, :],
                            in_=xf_ext[b:b + 1, :, :, :]
                                .transpose([1, 0, 2, 3]))
                        tile.add_dep_helper(xr.ins, ev1[2 * b].ins,
                                            reason="defer xres past L1")
                    glue((0, 2), (True, True))
                    glue((1, 3), (True, False))
                    ev2 = layer_fn(nc, psum, yt, w2, cst, 3, d, sumd2, sumps2)
                gstats2 = stats_fn(nc, pool, dram, sumd2, sumps2, "l2",
                                   no_collective)
                s2, t2 = _bn_affine(nc, pool, gstats2, cst[:, 2:4], g2, b2,
                                    eps, "l2")

                # ---- final: out = relu(s2*d + t2 + x), 128-wide, bf16 out;
                # DMA split across rings ----
                rings = [nc.sync, nc.gpsimd, nc.sync, nc.gpsimd,
                         nc.sync, nc.gpsimd, nc.scalar, nc.scalar]
                ri = 0
                for i in range(2):
                    for q in range(4):
                        rs = slice(14 * q, 14 * q + 14)
                        nc.vector.scalar_tensor_tensor(
                            out=d[:, i, rs, :], in0=d[:, i, rs, :],
                            scalar=s2[:, 0:1], in1=xres[:, i, rs, :],
                            op0=MULT, op1=ADD)
                        nc.scalar.activation(
                            out=dout[:, i, rs, :], in_=d[:, i, rs, :],
                            func=AF.Relu, bias=t2[:, 0:1], scale=1.0)
                        for half in range(2):
                            b = 2 * half + i
                            rings[ri % len(rings)].dma_start(
                                out=out_ext[b:b + 1, :, rs, :].transpose(
                                    [1, 0, 2, 3]),
                                in_=dout[64 * half:64 * half + C, i, rs, :])
                            ri += 1
    nc.compile()
    return nc


_NC_CACHE = None


def _get_nc():
    global _NC_CACHE
    if _NC_CACHE is None:
        _NC_CACHE = build()
    return _NC_CACHE


def _make_in_maps(x, centers1, gamma1, beta1, centers2, gamma2, beta2):
    from ml_dtypes import bfloat16

    def prep_w(centers):
        w = np.empty((2 * C, 9, C), np.float32)
        # centers: [o, d] with d = c*9 + k  ->  w[c, k, o] = -2*centers[o, 9c+k]
        w[:C] = -2.0 * np.ascontiguousarray(
            centers.reshape(C, C, 9).transpose(1, 2, 0))
        w[C:] = 1.0
        return w.astype(bfloat16)

    c1 = np.asarray(centers1, np.float32)
    c2 = np.asarray(centers2, np.float32)
    zero = np.zeros((C,), np.float32)
    # [zero, c2a, zero, c2b, g1, b1, g2, b2] so BN affine can fuse
    # [mu, E[d^2]] into one scalar_tensor_tensor against [zero, c2]
    cst = np.stack([
        zero, (c1 ** 2).sum(1), zero, (c2 ** 2).sum(1),
        np.asarray(gamma1, np.float32), np.asarray(beta1, np.float32),
        np.asarray(gamma2, np.float32), np.asarray(beta2, np.float32),
    ], axis=1).astype(np.float32)
    cst = np.ascontiguousarray(np.tile(cst, (2, 1)))   # duplicate both halves
    common = {
        "w1": prep_w(c1),
        "w2": prep_w(c2),
        "cst": cst,
    }
    xf = np.ascontiguousarray(np.asarray(x, np.float32))
    x = xf.astype(bfloat16)
    xsq = (xf * xf).astype(bfloat16)
    in_maps = []
    for c in range(N_CORES):
        m = dict(common)
        sl = slice(c * B_LOCAL, (c + 1) * B_LOCAL)
        m["x"] = np.ascontiguousarray(x[sl])
        m["xf"] = np.ascontiguousarray(xf[sl])
        m["xsq"] = np.ascontiguousarray(xsq[sl])
        in_maps.append(m)
    return in_maps


def _run(inputs, trace=False, **kw):
    nc = _get_nc()
    in_maps = _make_in_maps(**inputs)
    res = run_bass_kernel_spmd(nc, in_maps, core_ids=list(range(N_CORES)),
                               trace=trace, **kw)
    out = np.concatenate([res.results[c]["out"] for c in range(N_CORES)], axis=0)
    return out.astype(np.float32), res


def kernel(**inputs):
    out, _ = _run(inputs)
    return out


# revision 22
# speedup vs baseline: 1.1023x; 1.0046x over previous
"""Trainium2 Bass kernel for nn_BasicBlock (distance-transform conv BasicBlock).

Computes: relu(bn2(dt_conv2(relu(bn1(dt_conv1(x))))) + x)
where dt_conv is a 3x3 "distance transform conv":
    d[b,o,h,w] = sqrt(||p - c_o||^2),  p = 3x3 zero-padded patch (dim 576)

Strategy (8 NeuronCores, data-parallel over batch 32 -> 4 images/core):
- ||p||^2 - 2 p.c in ONE matmul accumulation group per pixel tile: SBUF
  partitions 0:64 hold x (weights = -2*centers), partitions 64:128 hold x^2
  (weights = 1.0). 9 shifted matmuls (3x3 offsets) accumulate in PSUM,
  K=128, M=64 out channels, N=448 (8 rows x 56).
- PE column-pairing: images (0,2) and (1,3) share one PSUM bank — image A
  accumulates into psum[0:64] (tile_position (0,0)), image B into
  psum[64:128] ((0,64)). The two 64-col PE groups run concurrently,
  ~doubling effective matmul throughput vs M=64 alone.
- All matmul operands bf16 (tolerance 2e-2; d and BN stats stay f32 where
  conditioning demands). Halves SBUF streams + input DMA.
- Evictions 128-wide: d = sqrt(psum + ||c||^2) on ScalarE per image-PAIR,
  accum_out gives per-channel sum(d). sum(d^2) = sum(psum) + n*c2 via one
  128-wide DVE reduce per pair.
- Sync-BN: fold upper/lower halves, [64,2] AllGather across 8 cores +
  local rank-sum, x2 layers.
- Output written bf16 and converted to f32 on host; out-DMA split across
  rings to shrink the post-barrier tail.

kernel(**inputs) takes FULL unsharded inputs, returns FULL output.
Self-contained: shapes/sharding hardcoded; no file reads.
"""
import numpy as np

from concourse import bacc, mybir, tile
from concourse.bass_utils import run_bass_kernel_spmd

f32 = mybir.dt.float32
bf16 = mybir.dt.bfloat16
ADD = mybir.AluOpType.add
MULT = mybir.AluOpType.mult
SUB = mybir.AluOpType.subtract
AF = mybir.ActivationFunctionType

N_CORES = 8
B_LOCAL = 4            # images per core (32 / 8)
C = 64                 # channels (in == out)
HW = 56                # spatial
HP = HW + 2            # padded
RPG = 8                # rows per matmul group (N = 8*56 = 448)
NGRP = 7               # row-groups per image-pair slot (56 / 8)
N_GLOBAL = 32 * HW * HW
BN_EPS = 1e-5


def _pb(b):
    """Partition base and pair-slot index for the 128-wide d/xres layout."""
    return 64 * (b // 2), b % 2


def _build_layer(nc, psum, src, w, cst, ci, d, sumd, sumps, slots=(0, 1)):
    """One dt_conv layer (or one pair-slot of it). src[b] is a [128, HP, HP]
    bf16 tile (x | x^2). Image pair (i, i+2) shares a PSUM bank: i ->
    psum[0:64] (PE cols 0:64), i+2 -> psum[64:128] (PE cols 64:128), running
    concurrently. d is [128, 2, HW, HW] f32; sumd/sumps [128, 2*NGRP].
    slots selects which image pairs to emit — callers phase slot 1 after
    its glue so ACT evictions interleave with glue instead of queueing
    behind it (ACT is strict FIFO; stuck evictions stall PSUM reuse)."""
    evicts = []
    order = [(g, i) for g in range(3) for i in slots] + \
            [(g, i) for g in range(3, NGRP) for i in slots]
    for g, i in order:
        ps = psum.tile([2 * C, RPG, 64], f32, tag="ps")
        r0 = g * RPG
        for k in range(9):
            kh, kw = k // 3, k % 3
            nc.tensor.matmul(
                ps[0:C, 0:RPG, 0:HW],
                w[:, k, :],
                src[i][:, r0 + kh:r0 + kh + RPG, kw:kw + HW],
                start=(k == 0), stop=(k == 8),
            )
            nc.tensor.matmul(
                ps[C:2 * C, 0:RPG, 0:HW],
                w[:, k, :],
                src[i + 2][:, r0 + kh:r0 + kh + RPG, kw:kw + HW],
                start=(k == 0), stop=(k == 8),
            )
        col = i * NGRP + g
        # per-channel sum(psum) (-> sum(d^2) after +n*c2); emitted first so
        # it runs concurrently with the ACT eviction
        nc.vector.tensor_reduce(
            out=sumps[:, col:col + 1],
            in_=ps[:, 0:RPG, 0:HW],
            axis=mybir.AxisListType.XY, op=ADD)
        # d = sqrt(psum + ||c||^2); accum_out gives per-channel sum(d)
        ev = nc.scalar.activation(
            out=d[:, i, r0:r0 + RPG, :],
            in_=ps[:, 0:RPG, 0:HW],
            func=AF.Sqrt, bias=cst[:, ci:ci + 1], scale=1.0,
            accum_out=sumd[:, col:col + 1])
        evicts.append(ev)
    return evicts


def _build_layer_nopair(nc, psum, src, w, cst, ci, d, sumd, sumps):
    """A/B variant: no PSUM column pairing — per-image groups, M=64 at
    tile_position (0,0), 64-wide evictions (baseline-style)."""
    evicts = []
    order = [(g, b) for g in range(3) for b in range(B_LOCAL)] + \
            [(g, b) for g in range(3, NGRP) for b in range(B_LOCAL)]
    for g, b in order:
        pb, i = _pb(b)
        ps = psum.tile([C, RPG, 64], f32, tag="psn")
        r0 = g * RPG
        for k in range(9):
            kh, kw = k // 3, k % 3
            nc.tensor.matmul(
                ps[:, 0:RPG, 0:HW],
                w[:, k, :],
                src[b][:, r0 + kh:r0 + kh + RPG, kw:kw + HW],
                start=(k == 0), stop=(k == 8),
            )
        col = b * NGRP + g
        nc.vector.tensor_reduce(
            out=sumps[:, col:col + 1],
            in_=ps[:, 0:RPG, 0:HW],
            axis=mybir.AxisListType.XY, op=ADD)
        ev = nc.scalar.activation(
            out=d[pb:pb + C, i, r0:r0 + RPG, :],
            in_=ps[:, 0:RPG, 0:HW],
            func=AF.Sqrt, bias=cst[pb:pb + C, ci:ci + 1], scale=1.0,
            accum_out=sumd[:, col:col + 1])
        evicts.append(ev)
    return evicts


def _stats_allreduce_nopair(nc, pool, dram, sumd, sumps, name,
                            no_collective=False):
    red = pool.tile([C, 2], f32, tag=f"redn_{name}")
    gstats = pool.tile([2 * C, 2], f32, tag=f"gstatsn_{name}")
    nc.vector.tensor_reduce(out=red[:, 0:1], in_=sumd[:, :],
                            axis=mybir.AxisListType.X, op=ADD)
    nc.vector.tensor_reduce(out=red[:, 1:2], in_=sumps[:, :],
                            axis=mybir.AxisListType.X, op=ADD)
    if no_collective:
        nc.vector.tensor_copy(out=gstats[0:C, :], in_=red[:, :])
        nc.vector.tensor_copy(out=gstats[C:2 * C, :], in_=gstats[0:C, :])
        return gstats
    cc_in = dram.tile([C, 2], f32, tag=f"ccinn_{name}")
    cc_out = dram.tile([N_CORES * C, 2], f32, tag=f"ccoutn_{name}")
    gag = pool.tile([C, N_CORES, 2], f32, tag=f"gagn_{name}")
    nc.sync.dma_start(out=cc_in[:, :], in_=red[:, :])
    nc.gpsimd.collective_compute(
        "AllGather", mybir.AluOpType.bypass,
        replica_groups=[list(range(N_CORES))],
        ins=[cc_in.opt()],
        outs=[cc_out.opt()],
    )
    nc.sync.dma_start(
        out=gag[:, :, :],
        in_=cc_out[:, :].rearrange("(r c) s -> c r s", r=N_CORES))
    nc.vector.tensor_reduce(out=gstats[0:C, 0:1], in_=gag[:, :, 0],
                            axis=mybir.AxisListType.X, op=ADD)
    nc.vector.tensor_reduce(out=gstats[0:C, 1:2], in_=gag[:, :, 1],
                            axis=mybir.AxisListType.X, op=ADD)
    nc.vector.tensor_copy(out=gstats[C:2 * C, :], in_=gstats[0:C, :])
    return gstats


def _bn_affine(nc, pool, gstats, zc2, gamma, beta, eps, name):
    """From [sum(d), sum(psum)] (dup both halves) -> scale s, shift t [128,1].
    zc2 is a [128, 2] cst slice [zero, c2]."""
    P = 2 * C
    mued = pool.tile([P, 2], f32, tag=f"mued_{name}")
    nvar = pool.tile([P, 1], f32, tag=f"nvar_{name}")
    sd = pool.tile([P, 1], f32, tag=f"sd_{name}")
    inv = pool.tile([P, 1], f32, tag=f"inv_{name}")
    s = pool.tile([P, 1], f32, tag=f"s_{name}")
    st = pool.tile([P, 1], f32, tag=f"st_{name}")
    tt = pool.tile([P, 1], f32, tag=f"t_{name}")
    inv_n = 1.0 / float(N_GLOBAL)
    # [mu, E[d^2]] = gstats * 1/N + [0, c2] in one DVE op
    nc.vector.scalar_tensor_tensor(
        out=mued[:, :], in0=gstats[:, 0:2], scalar=inv_n, in1=zc2,
        op0=MULT, op1=ADD)
    mu, ed2 = mued[:, 0:1], mued[:, 1:2]
    # -var = mu*mu - E[d^2] in one STT; sqrt flips the sign via scale=-1
    nc.vector.scalar_tensor_tensor(
        out=nvar[:, :], in0=mu, scalar=mu, in1=ed2, op0=MULT, op1=SUB)
    nc.scalar.activation(out=sd[:, :], in_=nvar[:, :], func=AF.Sqrt,
                         bias=eps[:, 0:1], scale=-1.0)
    nc.vector.reciprocal(out=inv[:, :], in_=sd[:, :])
    nc.vector.tensor_tensor(out=s[:, :], in0=gamma, in1=inv[:, :], op=MULT)
    nc.vector.tensor_tensor(out=st[:, :], in0=mu, in1=s[:, :], op=MULT)
    nc.vector.tensor_tensor(out=tt[:, :], in0=beta, in1=st[:, :], op=SUB)
    return s, tt


def _stats_allreduce(nc, pool, dram, sumd, sumps, name, no_collective=False):
    """Reduce [128, 2*NGRP] stat columns, fold upper half into lower,
    AllGather [64,2] across 8 cores + local rank-sum, return [128,2]
    duplicated global sums."""
    red = pool.tile([2 * C, 2], f32, tag=f"red_{name}")
    gstats = pool.tile([2 * C, 2], f32, tag=f"gstats_{name}")
    nc.vector.tensor_reduce(out=red[:, 0:1], in_=sumd[:, :],
                            axis=mybir.AxisListType.X, op=ADD)
    nc.vector.tensor_reduce(out=red[:, 1:2], in_=sumps[:, :],
                            axis=mybir.AxisListType.X, op=ADD)
    # images (2,3) stats live on the upper partition half; DMA both halves
    # side by side into the collective input (DVE tensor ops can't mix
    # partition bases, DMA can) and fold during the rank-sum reduce.
    cc_in = dram.tile([C, 2, 2], f32, tag=f"ccin_{name}")
    # one DMA: partition-major red [128,2] -> (h, c, s) walk of cc_in
    nc.sync.dma_start(out=cc_in[:, :, :].rearrange("c s h -> h c s"),
                      in_=red[:, :])
    if no_collective:
        gag = pool.tile([C, 2, 2], f32, tag=f"gag_{name}")
        nc.sync.dma_start(out=gag[:, :, :], in_=cc_in[:, :, :])
        nc.vector.tensor_reduce(out=gstats[0:C, 0:1], in_=gag[:, 0, :],
                                axis=mybir.AxisListType.X, op=ADD)
        nc.vector.tensor_reduce(out=gstats[0:C, 1:2], in_=gag[:, 1, :],
                                axis=mybir.AxisListType.X, op=ADD)
        nc.vector.tensor_copy(out=gstats[C:2 * C, :], in_=gstats[0:C, :])
        return gstats
    # AllGather (floor ~4.6us vs AllReduce ~9.7us) + local rank-sum.
    cc_out = dram.tile([N_CORES * C, 2, 2], f32, tag=f"ccout_{name}")
    gag = pool.tile([C, N_CORES, 2, 2], f32, tag=f"gag_{name}")
    nc.gpsimd.collective_compute(
        "AllGather", mybir.AluOpType.bypass,
        replica_groups=[list(range(N_CORES))],
        ins=[cc_in.opt()],
        outs=[cc_out.opt()],
    )
    nc.sync.dma_start(
        out=gag[:, :, :, :],
        in_=cc_out[:, :, :].rearrange("(r c) s h -> c r s h", r=N_CORES))
    nc.vector.tensor_reduce(out=gstats[0:C, 0:1], in_=gag[:, :, 0, :],
                            axis=mybir.AxisListType.XY, op=ADD)
    nc.vector.tensor_reduce(out=gstats[0:C, 1:2], in_=gag[:, :, 1, :],
                            axis=mybir.AxisListType.XY, op=ADD)
    nc.vector.tensor_copy(out=gstats[C:2 * C, :], in_=gstats[0:C, :])
    return gstats


def build(no_collective=False, reps=1, col_pair=True):
    nc = bacc.Bacc("TRN2", target_bir_lowering=False, debug=False,
                   num_devices=1 if no_collective else N_CORES)
    x_ext = nc.declare_dram_parameter("x", [B_LOCAL, C, HW, HW], bf16,
                                      isOutput=False)
    xf_ext = nc.declare_dram_parameter("xf", [B_LOCAL, C, HW, HW], f32,
                                       isOutput=False)
    xsq_ext = nc.declare_dram_parameter("xsq", [B_LOCAL, C, HW, HW], bf16,
                                        isOutput=False)
    w1_ext = nc.declare_dram_parameter("w1", [2 * C, 9, C], bf16, isOutput=False)
    w2_ext = nc.declare_dram_parameter("w2", [2 * C, 9, C], bf16, isOutput=False)
    # packed [c2a | c2b | g1 | b1 | g2 | b2], duplicated on both halves
    cst_ext = nc.declare_dram_parameter("cst", [2 * C, 8], f32, isOutput=False)
    out_ext = nc.declare_dram_parameter("out", [B_LOCAL, C, HW, HW], bf16,
                                        isOutput=True)

    with tile.TileContext(nc) as tc:
        with (
            tc.tile_pool(name="big", bufs=1) as big,
            tc.tile_pool(name="small", bufs=1) as pool,
            tc.tile_pool(name="psum", bufs=8, space="PSUM") as psum,
            tc.tile_pool(name="dram", bufs=1, space="DRAM") as dram,
        ):
            w1 = pool.tile([2 * C, 9, C], bf16, tag="w1")
            w2 = pool.tile([2 * C, 9, C], bf16, tag="w2")
            cst = pool.tile([2 * C, 8], f32, tag="cst")
            g1, b1 = cst[:, 4:5], cst[:, 5:6]
            g2, b2 = cst[:, 6:7], cst[:, 7:8]
            eps = pool.tile([2 * C, 1], f32, tag="eps")
            nc.vector.memset(eps[:, :], BN_EPS)
            # constants via the gpsimd SWDGE ring (SP/ACT rings carry x)
            nc.gpsimd.dma_start(out=w1[:, :, :], in_=w1_ext[:, :, :])
            nc.gpsimd.dma_start(out=cst[:, :], in_=cst_ext[:, :])
            nc.gpsimd.dma_start(out=w2[:, :, :], in_=w2_ext[:, :, :])

            for r in range(reps):
                xt = [big.tile([2 * C, HP, HP], bf16, tag=f"xt{b}",
                               name=f"xt{b}") for b in range(B_LOCAL)]
                yt = [big.tile([2 * C, HP, HP], bf16, tag=f"yt{b}",
                               name=f"yt{b}") for b in range(B_LOCAL)]
                # d + residual, 128-wide: partitions 0:64 = images 0,1;
                # 64:128 = images 2,3 (slot = b % 2)
                d = big.tile([2 * C, 2, HW, HW], f32, tag="d")
                xres = big.tile([2 * C, 2, HW, HW], f32, tag="xres")
                dout = big.tile([2 * C, 2, HW, HW], bf16, tag="dout")
                if col_pair:
                    sshape = [2 * C, 2 * NGRP]
                else:
                    sshape = [C, B_LOCAL * NGRP]
                sumd1 = pool.tile(sshape, f32, tag="sumd1")
                sumps1 = pool.tile(sshape, f32, tag="sumps1")
                sumd2 = pool.tile(sshape, f32, tag="sumd2")
                sumps2 = pool.tile(sshape, f32, tag="sumps2")
                layer_fn = _build_layer if col_pair else _build_layer_nopair
                stats_fn = (_stats_allreduce if col_pair
                            else _stats_allreduce_nopair)

                if r == 0:
                    # zero the pad borders once (interior-only writes after
                    # this keep them zero). x tiles on DVE, y tiles on Pool.
                    for tiles, eng in ((xt, nc.vector), (yt, nc.gpsimd)):
                        for t in tiles:
                            eng.memset(t[:, 0:1, :], 0.0)
                            eng.memset(t[:, HP - 1:HP, :], 0.0)
                            eng.memset(t[:, :, 0:1], 0.0)
                            eng.memset(t[:, :, HP - 1:HP], 0.0)

                # ---- x (sync ring) + host-computed x^2 (scalar ring) into
                # padded tiles, 2 chunks each ----
                for rows, pr in (((0, 32), (1, 33)), ((32, HW), (33, HW + 1))):
                    for b in (0, 2, 1, 3):
                        nc.sync.dma_start(
                            out=xt[b][0:C, pr[0]:pr[1], 1:HW + 1],
                            in_=x_ext[b:b + 1, :, rows[0]:rows[1], :]
                                .transpose([1, 0, 2, 3]))
                        nc.scalar.dma_start(
                            out=xt[b][C:2 * C, pr[0]:pr[1], 1:HW + 1],
                            in_=xsq_ext[b:b + 1, :, rows[0]:rows[1], :]
                                .transpose([1, 0, 2, 3]))

                # ---- layer 1 ----
                ev1 = layer_fn(nc, psum, xt, w1, cst, 1, d, sumd1, sumps1)

                # residual copy of x (f32 twin input; a casting DMA would
                # shatter into per-element descriptors), 128-wide layout;
                # needed only at the end, so defer past L1 start
                for b in range(B_LOCAL):
                    pb, i = _pb(b)
                    xr = nc.gpsimd.dma_start(
                        out=xres[pb:pb + C, i, :, :],
                        in_=xf_ext[b:b + 1, :, :, :].transpose([1, 0, 2, 3]))
                    tile.add_dep_helper(xr.ins, ev1[2 * b].ins,
                                        reason="defer xres DMA past L1 start")
                gstats1 = stats_fn(nc, pool, dram, sumd1, sumps1, "l1",
                                   no_collective)
                s1, t1 = _bn_affine(nc, pool, gstats1, cst[:, 0:2], g1, b1,
                                    eps, "l1")

                # ---- glue: y = relu(s1*d + t1) (bf16); y^2 on upper ----
                # 3 chunks; slot-0 glue, then L2 slot 0, then slot-1 glue
                # (interleaving with slot-0 evictions on ACT), then L2 slot 1
                gchunks = (((0, 30), (1, 31)), ((30, 56), (31, 57)))

                def glue(b_list, sq_engines):
                    for rows_d, rows_t in gchunks:
                        for b in b_list:
                            pb, i = _pb(b)
                            nc.scalar.activation(
                                out=yt[b][0:C, rows_t[0]:rows_t[1], 1:HW + 1],
                                in_=d[pb:pb + C, i, rows_d[0]:rows_d[1], :],
                                func=AF.Relu, bias=t1[pb:pb + C, 0:1],
                                scale=s1[pb:pb + C, 0:1])
                            sq_in = yt[b][0:C, rows_t[0]:rows_t[1], 1:HW + 1]
                            sq_out = yt[b][C:2 * C, rows_t[0]:rows_t[1],
                                           1:HW + 1]
                            if b in (0, 2):
                                nc.vector.tensor_tensor(
                                    out=sq_out, in0=sq_in, in1=sq_in, op=MULT)
                            else:
                                nc.scalar.activation(
                                    out=sq_out, in_=sq_in, func=AF.Square)

                # ---- layer 2 ----
                if col_pair:
                    glue((0, 2, 1, 3), None)
                    ev2 = _build_layer(nc, psum, yt, w2, cst, 3, d, sumd2,
                                       sumps2)
                else:
                    for b in range(B_LOCAL):
                        pb, i = _pb(b)
                        xr = nc.gpsimd.dma_start(
                            out=xres[pb:pb + C, i, :, :],
                            in_=xf_ext[b:b + 1, :, :, :]
                                .transpose([1, 0, 2, 3]))
                        tile.add_dep_helper(xr.ins, ev1[2 * b].ins,
                                            reason="defer xres past L1")
                    glue((0, 2), (True, True))
                    glue((1, 3), (True, False))
                    ev2 = layer_fn(nc, psum, yt, w2, cst, 3, d, sumd2, sumps2)
                gstats2 = stats_fn(nc, pool, dram, sumd2, sumps2, "l2",
                                   no_collective)
                s2, t2 = _bn_affine(nc, pool, gstats2, cst[:, 2:4], g2, b2,
                                    eps, "l2")

                # ---- final: out = relu(s2*d + t2 + x), 128-wide, bf16 out;
                # DMA split across rings ----
                rings = [nc.sync, nc.gpsimd, nc.sync, nc.gpsimd,
                         nc.sync, nc.gpsimd, nc.scalar, nc.scalar]
                ri = 0
                for i in range(2):
                    for q in range(4):
                        rs = slice(14 * q, 14 * q + 14)
                        nc.vector.scalar_tensor_tensor(
                            out=d[:, i, rs, :], in0=d[:, i, rs, :],
                            scalar=s2[:, 0:1], in1=xres[:, i, rs, :],
                            op0=MULT, op1=ADD)
                        nc.scalar.activation(
                            out=dout[:, i, rs, :], in_=d[:, i, rs, :],
                            func=AF.Relu, bias=t2[:, 0:1], scale=1.0)
                        for half in range(2):
                            b = 2 * half + i
                            rings[ri % len(rings)].dma_start(
                                out=out_ext[b:b + 1, :, rs, :].transpose(
                                    [1, 0, 2, 3]),
                                in_=dout[64 * half:64 * half + C, i, rs, :])
                            ri += 1
    nc.compile()
    return nc


_NC_CACHE = None


def _get_nc():
    global _NC_CACHE
    if _NC_CACHE is None:
        _NC_CACHE = build()
    return _NC_CACHE


def _make_in_maps(x, centers1, gamma1, beta1, centers2, gamma2, beta2):
    from ml_dtypes import bfloat16

    def prep_w(centers):
        w = np.empty((2 * C, 9, C), np.float32)
        # centers: [o, d] with d = c*9 + k  ->  w[c, k, o] = -2*centers[o, 9c+k]
        w[:C] = -2.0 * np.ascontiguousarray(
            centers.reshape(C, C, 9).transpose(1, 2, 0))
        w[C:] = 1.0
        return w.astype(bfloat16)

    c1 = np.asarray(centers1, np.float32)
    c2 = np.asarray(centers2, np.float32)
    zero = np.zeros((C,), np.float32)
    # [zero, c2a, zero, c2b, g1, b1, g2, b2] so BN affine can fuse
    # [mu, E[d^2]] into one scalar_tensor_tensor against [zero, c2]
    cst = np.stack([
        zero, (c1 ** 2).sum(1), zero, (c2 ** 2).sum(1),
        np.asarray(gamma1, np.float32), np.asarray(beta1, np.float32),
        np.asarray(gamma2, np.float32), np.asarray(beta2, np.float32),
    ], axis=1).astype(np.float32)
    cst = np.ascontiguousarray(np.tile(cst, (2, 1)))   # duplicate both halves
    common = {
        "w1": prep_w(c1),
        "w2": prep_w(c2),
        "cst": cst,
    }
    xf = np.ascontiguousarray(np.asarray(x, np.float32))
    x = xf.astype(bfloat16)
    xsq = (xf * xf).astype(bfloat16)
    in_maps = []
    for c in range(N_CORES):
        m = dict(common)
        sl = slice(c * B_LOCAL, (c + 1) * B_LOCAL)
        m["x"] = np.ascontiguousarray(x[sl])
        m["xf"] = np.ascontiguousarray(xf[sl])
        m["xsq"] = np.ascontiguousarray(xsq[sl])
        in_maps.append(m)
    return in_maps


def _run(inputs, trace=False, **kw):
    nc = _get_nc()
    in_maps = _make_in_maps(**inputs)
    res = run_bass_kernel_spmd(nc, in_maps, core_ids=list(range(N_CORES)),
                               trace=trace, **kw)
    out = np.concatenate([res.results[c]["out"] for c in range(N_CORES)], axis=0)
    return out.astype(np.float32), res


def kernel(**inputs):
    out, _ = _run(inputs)
    return out


# revision 23
# speedup vs baseline: 1.3006x; 1.1800x over previous
"""Trainium2 Bass kernel for nn_BasicBlock (distance-transform conv BasicBlock).

Computes: relu(bn2(dt_conv2(relu(bn1(dt_conv1(x))))) + x)
where dt_conv is a 3x3 "distance transform conv":
    d[b,o,h,w] = sqrt(||p - c_o||^2),  p = 3x3 zero-padded patch (dim 576)

Strategy (8 NeuronCores, data-parallel over batch 32 -> 4 images/core):
- ||p||^2 - 2 p.c in ONE matmul accumulation group per pixel tile: SBUF
  partitions 0:64 hold x (weights = -2*centers), partitions 64:128 hold x^2
  (weights = 1.0). 9 shifted matmuls (3x3 offsets) accumulate in PSUM,
  K=128, M=64 out channels, N=448 (8 rows x 56).
- PE column-pairing: images (0,2) and (1,3) share one PSUM bank — image A
  accumulates into psum[0:64] (tile_position (0,0)), image B into
  psum[64:128] ((0,64)). The two 64-col PE groups run concurrently,
  ~doubling effective matmul throughput vs M=64 alone.
- All matmul operands bf16 (tolerance 2e-2; d and BN stats stay f32 where
  conditioning demands). Halves SBUF streams + input DMA.
- Evictions 128-wide: d = sqrt(psum + ||c||^2) on ScalarE per image-PAIR,
  accum_out gives per-channel sum(d). sum(d^2) = sum(psum) + n*c2 via one
  128-wide DVE reduce per pair.
- Sync-BN: fold upper/lower halves, [64,2] AllGather across 8 cores +
  local rank-sum, x2 layers.
- Output written bf16 and converted to f32 on host; out-DMA split across
  rings to shrink the post-barrier tail.

kernel(**inputs) takes FULL unsharded inputs, returns FULL output.
Self-contained: shapes/sharding hardcoded; no file reads.
"""
import numpy as np

from concourse import bacc, mybir, tile
from concourse.bass_utils import run_bass_kernel_spmd

f32 = mybir.dt.float32
bf16 = mybir.dt.bfloat16
ADD = mybir.AluOpType.add
MULT = mybir.AluOpType.mult
SUB = mybir.AluOpType.subtract
AF = mybir.ActivationFunctionType

N_CORES = 8
B_LOCAL = 4            # images per core (32 / 8)
C = 64                 # channels (in == out)
HW = 56                # spatial
HP = HW + 2            # padded
RPG = 8                # rows per matmul group (N = 8*56 = 448)
NGRP = 7               # row-groups per image-pair slot (56 / 8)
N_GLOBAL = 32 * HW * HW
BN_EPS = 1e-5


def _pb(b):
    """Partition base and pair-slot index for the 128-wide d/xres layout."""
    return 64 * (b // 2), b % 2


def _build_layer(nc, psum, src, w, cst, ci, d, sumd, sumps, slots=(0, 1)):
    """One dt_conv layer (or one pair-slot of it). src[b] is a [128, HP, HP]
    bf16 tile (x | x^2). Image pair (i, i+2) shares a PSUM bank: i ->
    psum[0:64] (PE cols 0:64), i+2 -> psum[64:128] (PE cols 64:128), running
    concurrently. d is [128, 2, HW, HW] f32; sumd/sumps [128, 2*NGRP].
    slots selects which image pairs to emit — callers phase slot 1 after
    its glue so ACT evictions interleave with glue instead of queueing
    behind it (ACT is strict FIFO; stuck evictions stall PSUM reuse)."""
    evicts = []
    order = [(g, i) for g in range(3) for i in slots] + \
            [(g, i) for g in range(3, NGRP) for i in slots]
    for g, i in order:
        ps = psum.tile([2 * C, RPG, 64], f32, tag="ps")
        r0 = g * RPG
        for k in range(9):
            kh, kw = k // 3, k % 3
            nc.tensor.matmul(
                ps[0:C, 0:RPG, 0:HW],
                w[:, k, :],
                src[i][:, r0 + kh:r0 + kh + RPG, kw:kw + HW],
                start=(k == 0), stop=(k == 8),
            )
            nc.tensor.matmul(
                ps[C:2 * C, 0:RPG, 0:HW],
                w[:, k, :],
                src[i + 2][:, r0 + kh:r0 + kh + RPG, kw:kw + HW],
                start=(k == 0), stop=(k == 8),
            )
        col = i * NGRP + g
        # per-channel sum(psum) (-> sum(d^2) after +n*c2); emitted first so
        # it runs concurrently with the ACT eviction
        nc.vector.tensor_reduce(
            out=sumps[:, col:col + 1],
            in_=ps[:, 0:RPG, 0:HW],
            axis=mybir.AxisListType.XY, op=ADD)
        # d = sqrt(psum + ||c||^2); accum_out gives per-channel sum(d)
        ev = nc.scalar.activation(
            out=d[:, i, r0:r0 + RPG, :],
            in_=ps[:, 0:RPG, 0:HW],
            func=AF.Sqrt, bias=cst[:, ci:ci + 1], scale=1.0,
            accum_out=sumd[:, col:col + 1])
        evicts.append(ev)
    return evicts


def _build_layer_nopair(nc, psum, src, w, cst, ci, d, sumd, sumps):
    """A/B variant: no PSUM column pairing — per-image groups, M=64 at
    tile_position (0,0), 64-wide evictions (baseline-style)."""
    evicts = []
    order = [(g, b) for g in range(3) for b in range(B_LOCAL)] + \
            [(g, b) for g in range(3, NGRP) for b in range(B_LOCAL)]
    for g, b in order:
        pb, i = _pb(b)
        ps = psum.tile([C, RPG, 64], f32, tag="psn")
        r0 = g * RPG
        for k in range(9):
            kh, kw = k // 3, k % 3
            nc.tensor.matmul(
                ps[:, 0:RPG, 0:HW],
                w[:, k, :],
                src[b][:, r0 + kh:r0 + kh + RPG, kw:kw + HW],
                start=(k == 0), stop=(k == 8),
            )
        col = b * NGRP + g
        nc.vector.tensor_reduce(
            out=sumps[:, col:col + 1],
            in_=ps[:, 0:RPG, 0:HW],
            axis=mybir.AxisListType.XY, op=ADD)
        ev = nc.scalar.activation(
            out=d[pb:pb + C, i, r0:r0 + RPG, :],
            in_=ps[:, 0:RPG, 0:HW],
            func=AF.Sqrt, bias=cst[pb:pb + C, ci:ci + 1], scale=1.0,
            accum_out=sumd[:, col:col + 1])
        evicts.append(ev)
    return evicts


def _stats_allreduce_nopair(nc, pool, dram, sumd, sumps, name,
                            no_collective=False):
    red = pool.tile([C, 2], f32, tag=f"redn_{name}")
    gstats = pool.tile([2 * C, 2], f32, tag=f"gstatsn_{name}")
    nc.vector.tensor_reduce(out=red[:, 0:1], in_=sumd[:, :],
                            axis=mybir.AxisListType.X, op=ADD)
    nc.vector.tensor_reduce(out=red[:, 1:2], in_=sumps[:, :],
                            axis=mybir.AxisListType.X, op=ADD)
    if no_collective:
        nc.vector.tensor_copy(out=gstats[0:C, :], in_=red[:, :])
        nc.vector.tensor_copy(out=gstats[C:2 * C, :], in_=gstats[0:C, :])
        return gstats
    cc_in = dram.tile([C, 2], f32, tag=f"ccinn_{name}")
    cc_out = dram.tile([N_CORES * C, 2], f32, tag=f"ccoutn_{name}")
    gag = pool.tile([C, N_CORES, 2], f32, tag=f"gagn_{name}")
    nc.sync.dma_start(out=cc_in[:, :], in_=red[:, :])
    nc.gpsimd.collective_compute(
        "AllGather", mybir.AluOpType.bypass,
        replica_groups=[list(range(N_CORES))],
        ins=[cc_in.opt()],
        outs=[cc_out.opt()],
    )
    nc.sync.dma_start(
        out=gag[:, :, :],
        in_=cc_out[:, :].rearrange("(r c) s -> c r s", r=N_CORES))
    nc.vector.tensor_reduce(out=gstats[0:C, 0:1], in_=gag[:, :, 0],
                            axis=mybir.AxisListType.X, op=ADD)
    nc.vector.tensor_reduce(out=gstats[0:C, 1:2], in_=gag[:, :, 1],
                            axis=mybir.AxisListType.X, op=ADD)
    nc.vector.tensor_copy(out=gstats[C:2 * C, :], in_=gstats[0:C, :])
    return gstats


def _bn_affine(nc, pool, gstats, zc2, gamma, beta, eps, name):
    """From [sum(d), sum(psum)] (dup both halves) -> scale s, shift t [128,1].
    zc2 is a [128, 2] cst slice [zero, c2]."""
    P = 2 * C
    mued = pool.tile([P, 2], f32, tag=f"mued_{name}")
    nvar = pool.tile([P, 1], f32, tag=f"nvar_{name}")
    sd = pool.tile([P, 1], f32, tag=f"sd_{name}")
    inv = pool.tile([P, 1], f32, tag=f"inv_{name}")
    s = pool.tile([P, 1], f32, tag=f"s_{name}")
    st = pool.tile([P, 1], f32, tag=f"st_{name}")
    tt = pool.tile([P, 1], f32, tag=f"t_{name}")
    inv_n = 1.0 / float(N_GLOBAL)
    # [mu, E[d^2]] = gstats * 1/N + [0, c2] in one DVE op
    nc.vector.scalar_tensor_tensor(
        out=mued[:, :], in0=gstats[:, 0:2], scalar=inv_n, in1=zc2,
        op0=MULT, op1=ADD)
    mu, ed2 = mued[:, 0:1], mued[:, 1:2]
    # -var = mu*mu - E[d^2] in one STT; sqrt flips the sign via scale=-1
    nc.vector.scalar_tensor_tensor(
        out=nvar[:, :], in0=mu, scalar=mu, in1=ed2, op0=MULT, op1=SUB)
    nc.scalar.activation(out=sd[:, :], in_=nvar[:, :], func=AF.Sqrt,
                         bias=eps[:, 0:1], scale=-1.0)
    nc.vector.reciprocal(out=inv[:, :], in_=sd[:, :])
    nc.vector.tensor_tensor(out=s[:, :], in0=gamma, in1=inv[:, :], op=MULT)
    nc.vector.tensor_tensor(out=st[:, :], in0=mu, in1=s[:, :], op=MULT)
    nc.vector.tensor_tensor(out=tt[:, :], in0=beta, in1=st[:, :], op=SUB)
    return s, tt


def _stats_allreduce(nc, pool, dram, sumd, sumps, name, no_collective=False):
    """Reduce [128, 2*NGRP] stat columns, fold upper half into lower,
    AllGather [64,2] across 8 cores + local rank-sum, return [128,2]
    duplicated global sums."""
    red = pool.tile([2 * C, 2], f32, tag=f"red_{name}")
    gstats = pool.tile([2 * C, 2], f32, tag=f"gstats_{name}")
    nc.vector.tensor_reduce(out=red[:, 0:1], in_=sumd[:, :],
                            axis=mybir.AxisListType.X, op=ADD)
    nc.vector.tensor_reduce(out=red[:, 1:2], in_=sumps[:, :],
                            axis=mybir.AxisListType.X, op=ADD)
    # images (2,3) stats live on the upper partition half; DMA both halves
    # side by side into the collective input (DVE tensor ops can't mix
    # partition bases, DMA can) and fold during the rank-sum reduce.
    cc_in = dram.tile([C, 2, 2], f32, tag=f"ccin_{name}")
    # one DMA: partition-major red [128,2] -> (h, c, s) walk of cc_in
    nc.sync.dma_start(out=cc_in[:, :, :].rearrange("c s h -> h c s"),
                      in_=red[:, :])
    if no_collective:
        gag = pool.tile([C, 2, 2], f32, tag=f"gag_{name}")
        nc.sync.dma_start(out=gag[:, :, :], in_=cc_in[:, :, :])
        nc.vector.tensor_reduce(out=gstats[0:C, 0:1], in_=gag[:, 0, :],
                                axis=mybir.AxisListType.X, op=ADD)
        nc.vector.tensor_reduce(out=gstats[0:C, 1:2], in_=gag[:, 1, :],
                                axis=mybir.AxisListType.X, op=ADD)
        nc.vector.tensor_copy(out=gstats[C:2 * C, :], in_=gstats[0:C, :])
        return gstats
    # AllGather (floor ~4.6us vs AllReduce ~9.7us) + local rank-sum.
    cc_out = dram.tile([N_CORES * C, 2, 2], f32, tag=f"ccout_{name}")
    gag = pool.tile([C, N_CORES, 2, 2], f32, tag=f"gag_{name}")
    nc.gpsimd.collective_compute(
        "AllGather", mybir.AluOpType.bypass,
        replica_groups=[list(range(N_CORES))],
        ins=[cc_in.opt()],
        outs=[cc_out.opt()],
    )
    nc.sync.dma_start(
        out=gag[:, :, :, :],
        in_=cc_out[:, :, :].rearrange("(r c) s h -> c r s h", r=N_CORES))
    nc.vector.tensor_reduce(out=gstats[0:C, 0:1], in_=gag[:, :, 0, :],
                            axis=mybir.AxisListType.XY, op=ADD)
    nc.vector.tensor_reduce(out=gstats[0:C, 1:2], in_=gag[:, :, 1, :],
                            axis=mybir.AxisListType.XY, op=ADD)
    nc.vector.tensor_copy(out=gstats[C:2 * C, :], in_=gstats[0:C, :])
    return gstats


def build(no_collective=False, reps=1, col_pair=True):
    nc = bacc.Bacc("TRN2", target_bir_lowering=False, debug=False,
                   num_devices=1 if no_collective else N_CORES)
    x_ext = nc.declare_dram_parameter("x", [B_LOCAL, C, HW, HW], bf16,
                                      isOutput=False)
    xf_ext = nc.declare_dram_parameter("xf", [B_LOCAL, C, HW, HW], f32,
                                       isOutput=False)
    xsq_ext = nc.declare_dram_parameter("xsq", [B_LOCAL, C, HW, HW], bf16,
                                        isOutput=False)
    w1_ext = nc.declare_dram_parameter("w1", [2 * C, 9, C], bf16, isOutput=False)
    w2_ext = nc.declare_dram_parameter("w2", [2 * C, 9, C], bf16, isOutput=False)
    # packed [c2a | c2b | g1 | b1 | g2 | b2], duplicated on both halves
    cst_ext = nc.declare_dram_parameter("cst", [2 * C, 8], f32, isOutput=False)
    out_ext = nc.declare_dram_parameter("out", [B_LOCAL, C, HW, HW], bf16,
                                        isOutput=True)

    with tile.TileContext(nc) as tc:
        with (
            tc.tile_pool(name="big", bufs=1) as big,
            tc.tile_pool(name="small", bufs=1) as pool,
            tc.tile_pool(name="psum", bufs=8, space="PSUM") as psum,
            tc.tile_pool(name="dram", bufs=1, space="DRAM") as dram,
        ):
            w1 = pool.tile([2 * C, 9, C], bf16, tag="w1")
            w2 = pool.tile([2 * C, 9, C], bf16, tag="w2")
            cst = pool.tile([2 * C, 8], f32, tag="cst")
            g1, b1 = cst[:, 4:5], cst[:, 5:6]
            g2, b2 = cst[:, 6:7], cst[:, 7:8]
            eps = pool.tile([2 * C, 1], f32, tag="eps")
            nc.vector.memset(eps[:, :], BN_EPS)
            # constants via the gpsimd SWDGE ring (SP/ACT rings carry x)
            nc.gpsimd.dma_start(out=w1[:, :, :], in_=w1_ext[:, :, :])
            nc.gpsimd.dma_start(out=cst[:, :], in_=cst_ext[:, :])
            nc.gpsimd.dma_start(out=w2[:, :, :], in_=w2_ext[:, :, :])

            for r in range(reps):
                xt = [big.tile([2 * C, HP, HP], bf16, tag=f"xt{b}",
                               name=f"xt{b}") for b in range(B_LOCAL)]
                yt = [big.tile([2 * C, HP, HP], bf16, tag=f"yt{b}",
                               name=f"yt{b}") for b in range(B_LOCAL)]
                # d + residual, 128-wide: partitions 0:64 = images 0,1;
                # 64:128 = images 2,3 (slot = b % 2)
                d = big.tile([2 * C, 2, HW, HW], f32, tag="d")
                xres = big.tile([2 * C, 2, HW, HW], f32, tag="xres")
                dout = big.tile([2 * C, 2, HW, HW], bf16, tag="dout")
                if col_pair:
                    sshape = [2 * C, 2 * NGRP]
                else:
                    sshape = [C, B_LOCAL * NGRP]
                sumd1 = pool.tile(sshape, f32, tag="sumd1")
                sumps1 = pool.tile(sshape, f32, tag="sumps1")
                sumd2 = pool.tile(sshape, f32, tag="sumd2")
                sumps2 = pool.tile(sshape, f32, tag="sumps2")
                layer_fn = _build_layer if col_pair else _build_layer_nopair
                stats_fn = (_stats_allreduce if col_pair
                            else _stats_allreduce_nopair)

                if r == 0:
                    # zero the pad borders once (interior-only writes after
                    # this keep them zero). x tiles on DVE, y tiles on Pool.
                    for tiles, eng in ((xt, nc.vector), (yt, nc.gpsimd)):
                        for t in tiles:
                            eng.memset(t[:, 0:1, :], 0.0)
                            eng.memset(t[:, HP - 1:HP, :], 0.0)
                            eng.memset(t[:, :, 0:1], 0.0)
                            eng.memset(t[:, :, HP - 1:HP], 0.0)

                # ---- x (sync ring) + host-computed x^2 (scalar ring) into
                # padded tiles, 2 chunks each ----
                for rows, pr in (((0, 32), (1, 33)), ((32, HW), (33, HW + 1))):
                    for b in (0, 2, 1, 3):
                        nc.sync.dma_start(
                            out=xt[b][0:C, pr[0]:pr[1], 1:HW + 1],
                            in_=x_ext[b:b + 1, :, rows[0]:rows[1], :]
                                .transpose([1, 0, 2, 3]))
                        nc.scalar.dma_start(
                            out=xt[b][C:2 * C, pr[0]:pr[1], 1:HW + 1],
                            in_=xsq_ext[b:b + 1, :, rows[0]:rows[1], :]
                                .transpose([1, 0, 2, 3]))

                # ---- layer 1 ----
                ev1 = layer_fn(nc, psum, xt, w1, cst, 1, d, sumd1, sumps1)

                # residual copy of x (f32 twin input; a casting DMA would
                # shatter into per-element descriptors), 128-wide layout;
                # needed only at the end, so defer past L1 start
                for b in range(B_LOCAL):
                    pb, i = _pb(b)
                    xr = nc.gpsimd.dma_start(
                        out=xres[pb:pb + C, i, :, :],
                        in_=xf_ext[b:b + 1, :, :, :].transpose([1, 0, 2, 3]))
                    tile.add_dep_helper(xr.ins, ev1[2 * b].ins,
                                        reason="defer xres DMA past L1 start")
                gstats1 = stats_fn(nc, pool, dram, sumd1, sumps1, "l1",
                                   no_collective)
                s1, t1 = _bn_affine(nc, pool, gstats1, cst[:, 0:2], g1, b1,
                                    eps, "l1")

                # ---- glue: y = relu(s1*d + t1) (bf16); y^2 on upper ----
                # 3 chunks; slot-0 glue, then L2 slot 0, then slot-1 glue
                # (interleaving with slot-0 evictions on ACT), then L2 slot 1
                # first chunk sized to unblock L2 g0/g1 fast (needs 18
                # y-rows); PE restart after barrier-1 gates the whole rep
                gchunks = (((0, 18), (1, 19)), ((18, 37), (19, 38)),
                           ((37, 56), (38, 57)))

                def glue(b_list, sq_engines):
                    for rows_d, rows_t in gchunks:
                        for b in b_list:
                            pb, i = _pb(b)
                            nc.scalar.activation(
                                out=yt[b][0:C, rows_t[0]:rows_t[1], 1:HW + 1],
                                in_=d[pb:pb + C, i, rows_d[0]:rows_d[1], :],
                                func=AF.Relu, bias=t1[pb:pb + C, 0:1],
                                scale=s1[pb:pb + C, 0:1])
                            sq_in = yt[b][0:C, rows_t[0]:rows_t[1], 1:HW + 1]
                            sq_out = yt[b][C:2 * C, rows_t[0]:rows_t[1],
                                           1:HW + 1]
                            if b in (0, 2):
                                nc.vector.tensor_tensor(
                                    out=sq_out, in0=sq_in, in1=sq_in, op=MULT)
                            else:
                                nc.scalar.activation(
                                    out=sq_out, in_=sq_in, func=AF.Square)

                # ---- layer 2 ----
                if col_pair:
                    glue((0, 2, 1, 3), None)
                    ev2 = _build_layer(nc, psum, yt, w2, cst, 3, d, sumd2,
                                       sumps2)
                else:
                    for b in range(B_LOCAL):
                        pb, i = _pb(b)
                        xr = nc.gpsimd.dma_start(
                            out=xres[pb:pb + C, i, :, :],
                            in_=xf_ext[b:b + 1, :, :, :]
                                .transpose([1, 0, 2, 3]))
                        tile.add_dep_helper(xr.ins, ev1[2 * b].ins,
                                            reason="defer xres past L1")
                    glue((0, 2), (True, True))
                    glue((1, 3), (True, False))
                    ev2 = layer_fn(nc, psum, yt, w2, cst, 3, d, sumd2, sumps2)
                gstats2 = stats_fn(nc, pool, dram, sumd2, sumps2, "l2",
                                   no_collective)
                s2, t2 = _bn_affine(nc, pool, gstats2, cst[:, 2:4], g2, b2,
                                    eps, "l2")

                # ---- final: out = relu(s2*d + t2 + x), 128-wide, bf16 out;
                # DMA split across rings ----
                rings = [nc.sync, nc.gpsimd, nc.sync, nc.gpsimd,
                         nc.sync, nc.gpsimd, nc.scalar, nc.scalar]
                ri = 0
                for i in range(2):
                    for q in range(4):
                        rs = slice(14 * q, 14 * q + 14)
                        nc.vector.scalar_tensor_tensor(
                            out=d[:, i, rs, :], in0=d[:, i, rs, :],
                            scalar=s2[:, 0:1], in1=xres[:, i, rs, :],
                            op0=MULT, op1=ADD)
                        nc.scalar.activation(
                            out=dout[:, i, rs, :], in_=d[:, i, rs, :],
                            func=AF.Relu, bias=t2[:, 0:1], scale=1.0)
                        for half in range(2):
                            b = 2 * half + i
                            rings[ri % len(rings)].dma_start(
                                out=out_ext[b:b + 1, :, rs, :].transpose(
                                    [1, 0, 2, 3]),
                                in_=dout[64 * half:64 * half + C, i, rs, :])
                            ri += 1
    nc.compile()
    return nc


_NC_CACHE = None


def _get_nc():
    global _NC_CACHE
    if _NC_CACHE is None:
        _NC_CACHE = build()
    return _NC_CACHE


def _make_in_maps(x, centers1, gamma1, beta1, centers2, gamma2, beta2):
    from ml_dtypes import bfloat16

    def prep_w(centers):
        w = np.empty((2 * C, 9, C), np.float32)
        # centers: [o, d] with d = c*9 + k  ->  w[c, k, o] = -2*centers[o, 9c+k]
        w[:C] = -2.0 * np.ascontiguousarray(
            centers.reshape(C, C, 9).transpose(1, 2, 0))
        w[C:] = 1.0
        return w.astype(bfloat16)

    c1 = np.asarray(centers1, np.float32)
    c2 = np.asarray(centers2, np.float32)
    zero = np.zeros((C,), np.float32)
    # [zero, c2a, zero, c2b, g1, b1, g2, b2] so BN affine can fuse
    # [mu, E[d^2]] into one scalar_tensor_tensor against [zero, c2]
    cst = np.stack([
        zero, (c1 ** 2).sum(1), zero, (c2 ** 2).sum(1),
        np.asarray(gamma1, np.float32), np.asarray(beta1, np.float32),
        np.asarray(gamma2, np.float32), np.asarray(beta2, np.float32),
    ], axis=1).astype(np.float32)
    cst = np.ascontiguousarray(np.tile(cst, (2, 1)))   # duplicate both halves
    common = {
        "w1": prep_w(c1),
        "w2": prep_w(c2),
        "cst": cst,
    }
    xf = np.ascontiguousarray(np.asarray(x, np.float32))
    x = xf.astype(bfloat16)
    xsq = (xf * xf).astype(bfloat16)
    in_maps = []
    for c in range(N_CORES):
        m = dict(common)
        sl = slice(c * B_LOCAL, (c + 1) * B_LOCAL)
        m["x"] = np.ascontiguousarray(x[sl])
        m["xf"] = np.ascontiguousarray(xf[sl])
        m["xsq"] = np.ascontiguousarray(xsq[sl])
        in_maps.append(m)
    return in_maps


def _run(inputs, trace=False, **kw):
    nc = _get_nc()
    in_maps = _make_in_maps(**inputs)
    res = run_bass_kernel_spmd(nc, in_maps, core_ids=list(range(N_CORES)),
                               trace=trace, **kw)
    out = np.concatenate([res.results[c]["out"] for c in range(N_CORES)], axis=0)
    return out.astype(np.float32), res


def kernel(**inputs):
    out, _ = _run(inputs)
    return out
